# revision 1
# baseline (speedup 1.0000x reference)
"""GATv2 (3-layer, 8-head) on 8 Trainium2 NeuronCores.

Strategy (edge-parallel, dst-sharded):
- Core c owns destination nodes [c*N/8, (c+1)*N/8) and all edges into them.
- Host sorts each core's edges by (dst-window, src-half), pads to 128-edge
  chunks with a chunk structure made uniform across cores (SPMD: one program).
- Per layer: fs = h@Wsrc for ALL nodes (layer 0: replicated GEMM from the
  replicated features; layers 1/2: sharded GEMM + AllGather), fd = h@Wdst for
  the local shard only.
- Edge phase per 128-dst window: dma_gather fs[src] rows (the only per-edge
  gather), expand fd[dst] via one-hot matmul, score
  s = attn . leaky_relu(fs+fd) via DVE mul+reduce, ex = exp(s) (no segment-max:
  scores are O(1) so exp is safe), unnormalized aggregation
  rstU = OneHot @ [ex*z | ex] via TensorE (denominator rides in the last 8
  columns), then per-window normalization rst = rstU/denom - fd (using
  sum_e ex*fd[dst] = denom*fd[v]) + residual, relu.
- Output: mean over heads, host concatenates the 8 dst shards.
"""
import sys
sys.path.insert(0, "/opt/trn_rl_repo")
import numpy as np
import concourse.bass as bass
import concourse.mybir as mybir
import concourse.tile as tile
from concourse import bacc
from concourse.bass_utils import run_bass_kernel_spmd

P = 128
NCORE = 8
SLOPE = 0.2
H = 8

F32 = mybir.dt.float32
BF16 = mybir.dt.bfloat16
I16 = mybir.dt.int16
AX = mybir.AxisListType
OP = mybir.AluOpType
AF = mybir.ActivationFunctionType


# ---------------------------------------------------------------- host layout
def build_layout(src, dst, N):
    """Edge layout: per-core, dst-window-sorted, src-half-split, 128-padded,
    chunk structure uniform across cores."""
    SH = N // NCORE
    NW = (SH + P - 1) // P
    HALF = min(32768, (N + 1) // 2)  # src-half split point (int16 idx range)
    cores = []
    for c in range(NCORE):
        m = (dst // SH) == c
        s, d = src[m], dst[m]
        dl = d - c * SH
        w = dl // P
        hf = (s >= HALF).astype(np.int64)
        order = np.lexsort((hf, w))
        s, dl, w, hf = s[order], dl[order], w[order], hf[order]
        groups = {}
        for wi in range(NW):
            for h in range(2):
                gm = (w == wi) & (hf == h)
                groups[(wi, h)] = (s[gm], dl[gm])
        cores.append(groups)
    # uniform chunk counts per (window, half)
    C = {}
    for wi in range(NW):
        for h in range(2):
            n = max(len(cores[c][(wi, h)][0]) for c in range(NCORE))
            C[(wi, h)] = (n + P - 1) // P
    NCH = sum(C.values())
    # flat edge arrays per core
    src_rel = np.zeros((NCORE, NCH * P), np.int16)
    dstw = np.full((NCORE, NCH * P), -1.0, np.float32)
    calls = []  # (half, chunk_off, n_chunks) shared structure
    off = 0
    for wi in range(NW):
        for h in range(2):
            nch = C[(wi, h)]
            g = 0
            while g < nch:
                gs = min(8, nch - g)
                calls.append((wi, h, off + g, gs))
                g += gs
            for c in range(NCORE):
                s, dl = cores[c][(wi, h)]
                k = off * P
                src_rel[c, k:k + len(s)] = (s - h * HALF).astype(np.int16)
                dstw[c, k:k + len(dl)] = (dl % P).astype(np.float32)
            off += nch
    assert off == NCH
    # wrapped int16 idx for dma_gather: per call, idx i -> [i%16, col+i//16]
    TOTC = NCH * P // 16
    idx_w = np.zeros((NCORE, P, TOTC), np.int16)
    for c in range(NCORE):
        w16 = src_rel[c].reshape(-1, 16).T  # [16, NCH*8]
        idx_w[c] = np.tile(w16, (8, 1))
    # chunk-column-major dstw: [P, NCH], edge k*P+p -> [p, k]
    dstw_cols = dstw.reshape(NCORE, NCH, P).transpose(0, 2, 1).copy()
    return dict(SH=SH, NW=NW, HALF=HALF, NCH=NCH, C=C, calls=calls,
                idx_w=idx_w, dstw_cols=dstw_cols)


# ---------------------------------------------------------------- bass kernel
def build_kernel(N, IN, L):
    """L = layout dict. IN = input feature dim (128)."""
    D = 256  # H*HID = H*OUT
    SH, NW, NCH = L["SH"], L["NW"], L["NCH"]
    HALF = L["HALF"]
    T0 = (N + P - 1) // P            # full-N tiles (layer-0 fs GEMM)
    SHP = NW * P                      # padded shard rows
    TOTC = NCH * P // 16

    nc = bacc.Bacc("TRN2", target_bir_lowering=False, debug=False,
                   num_devices=NCORE)
    # ---- inputs (per-core where noted)
    featT = nc.declare_dram_parameter("featT", [IN, N], F32, isOutput=False)
    featT_loc = nc.declare_dram_parameter("featT_loc", [IN, SH], F32, isOutput=False)
    Ws = [nc.declare_dram_parameter(f"Wsrc{l}", [IN if l == 0 else D, D], F32, isOutput=False) for l in range(3)]
    Wd = [nc.declare_dram_parameter(f"Wdst{l}", [IN if l == 0 else D, D], F32, isOutput=False) for l in range(3)]
    Wres1 = nc.declare_dram_parameter("Wres1", [IN, D], F32, isOutput=False)
    attn4 = [nc.declare_dram_parameter(f"attn4_{l}", [P, 4 * D], F32, isOutput=False) for l in range(3)]
    iota4 = nc.declare_dram_parameter("iota4", [P, 4 * P], F32, isOutput=False)
    ident_in = nc.declare_dram_parameter("ident", [P, P], F32, isOutput=False)
    idx_in = nc.declare_dram_parameter("idx_w", [P, TOTC], I16, isOutput=False)   # per-core
    dstw_in = nc.declare_dram_parameter("dstw", [P, NCH], F32, isOutput=False)    # per-core
    out_ext = nc.declare_dram_parameter("out", [SH, 32], F32, isOutput=True)

    with tile.TileContext(nc) as tc:
        with (
            tc.tile_pool(name="const", bufs=1) as cpool,
            tc.tile_pool(name="sbuf", bufs=4) as sb,
            tc.tile_pool(name="sb6", bufs=4) as sb6,
            tc.tile_pool(name="sb2", bufs=3) as sb2,
            tc.tile_pool(name="psum", bufs=2, space="PSUM") as ps,
            tc.tile_pool(name="dram", bufs=1, space="DRAM") as dr,
        ):
            # ---- persistent constants
            ident = cpool.tile([P, P], F32)
            nc.sync.dma_start(out=ident[:], in_=ident_in[:, :])
            ident16 = cpool.tile([P, P], BF16, tag="ident16")
            nc.vector.tensor_copy(out=ident16[:], in_=ident[:])
            iota_t = cpool.tile([P, 4 * P], F32)
            nc.sync.dma_start(out=iota_t[:], in_=iota4[:, :])
            idx_t = cpool.tile([P, TOTC], I16)
            nc.sync.dma_start(out=idx_t[:], in_=idx_in[:, :])
            dstw_t = cpool.tile([P, NCH], F32)
            nc.sync.dma_start(out=dstw_t[:], in_=dstw_in[:, :])
            attn_t = []
            for l in range(3):
                a32 = cpool.tile([P, 4 * D], F32, tag=f"attn32_{l}")
                nc.sync.dma_start(out=a32[:], in_=attn4[l][:, :])
                a = cpool.tile([P, 4 * D], BF16, tag=f"attn{l}")
                nc.vector.tensor_copy(out=a[:], in_=a32[:])
                attn_t.append(a)
            hT = cpool.tile([P, 2, SHP], F32, tag="hT")  # local shard, transposed

            # ---- DRAM internals
            fs_full0 = dr.tile([T0 * P, D], BF16, tag="fsf0")
            fs_full1 = dr.tile([N, D], BF16, tag="fsf1", addr_space="Shared")
            fs_full2 = dr.tile([N, D], BF16, tag="fsf2", addr_space="Shared")
            fs_full_l = [None, fs_full1, fs_full2]
            ag_in = dr.tile([SH, D], BF16, tag="agin")
            fd_dram = dr.tile([SHP, D], BF16, tag="fd")
            res_dram = dr.tile([SHP, D], F32, tag="res")
            h_dram = dr.tile([SHP, D], F32, tag="h")

            zero_sb = cpool.tile([P, D], F32, tag="zero")
            nc.vector.memset(zero_sb[:], 0.0)
            zero16 = cpool.tile([P, D], BF16, tag="zero16")
            nc.vector.memset(zero16[:], 0.0)
            if SHP > SH:  # zero the padded tails once
                nc.sync.dma_start(out=fd_dram[SH:SHP, :], in_=zero16[:SHP - SH, :])
                nc.sync.dma_start(out=res_dram[SH:SHP, :], in_=zero_sb[:SHP - SH, :])
                nc.sync.dma_start(out=h_dram[SH:SHP, :], in_=zero_sb[:SHP - SH, :])

            def gemm(lhsT_ap_fn, kchunks, rhs_t, mt, out_psum):
                """out_psum[mt, D] = sum_k lhsT_k.T @ rhs_k"""
                for k in range(kchunks):
                    nc.tensor.matmul(out_psum[:mt, :D], lhsT=lhsT_ap_fn(k),
                                     rhs=rhs_t[:, k, :],
                                     start=(k == 0), stop=(k == kchunks - 1))

            def load_w(wparam, kchunks, tag):
                wt = cpool.tile([P, kchunks, D], F32, tag=tag)
                nc.sync.dma_start(
                    out=wt[:], in_=wparam.ap().rearrange("(c k) n -> k c n", k=P))
                return wt

            wsrc_t = [load_w(Ws[l], 1 if l == 0 else 2, f"wsrc{l}") for l in range(3)]
            wdst_t = [load_w(Wd[l], 1 if l == 0 else 2, f"wdst{l}") for l in range(3)]
            wres_t = load_w(Wres1, 1, "wres")

            for l in range(3):
                DIN = IN if l == 0 else D
                KCH = DIN // P
                act_relu = l < 2
                # ================= dense phase =================
                if l == 0:
                    # replicated fs_full GEMM from featT
                    for t in range(T0):
                        mt = min(P, N - t * P)
                        lt = sb.tile([P, P], F32, tag="lhsT")
                        nc.sync.dma_start(out=lt[:, :mt], in_=featT[:, t * P:t * P + mt])
                        pst = ps.tile([P, 264], F32, tag="rstcat", space="PSUM")
                        nc.tensor.matmul(pst[:mt, :D], lhsT=lt[:, :mt],
                                         rhs=wsrc_t[0][:, 0, :], start=True, stop=True)
                        ot = sb.tile([P, D], BF16, tag="gout")
                        nc.scalar.copy(out=ot[:mt, :], in_=pst[:mt, :D])
                        nc.sync.dma_start(out=fs_full0[t * P:t * P + mt, :], in_=ot[:mt, :])
                    # sharded fd / res GEMMs from featT_loc
                    for w in range(NW):
                        mt = min(P, SH - w * P)
                        lt = sb.tile([P, P], F32, tag="lhsT")
                        nc.sync.dma_start(out=lt[:, :mt], in_=featT_loc[:, w * P:w * P + mt])
                        for (rhs_t, dest, dt_) in ((wdst_t[0], fd_dram, BF16), (wres_t, res_dram, F32)):
                            pst = ps.tile([P, 264], F32, tag="rstcat", space="PSUM")
                            nc.tensor.matmul(pst[:mt, :D], lhsT=lt[:, :mt],
                                             rhs=rhs_t[:, 0, :], start=True, stop=True)
                            ot = sb.tile([P, D], dt_, tag="gout" if dt_ is BF16 else "gout32")
                            nc.scalar.copy(out=ot[:mt, :], in_=pst[:mt, :D])
                            nc.sync.dma_start(out=dest[w * P:w * P + mt, :], in_=ot[:mt, :])
                else:
                    # sharded fs -> ag_in; fd -> fd_dram (lhsT = resident hT)
                    for w in range(NW):
                        mt = min(P, SH - w * P)
                        for (rhs_t, dest) in ((wsrc_t[l], ag_in), (wdst_t[l], fd_dram)):
                            pst = ps.tile([P, 264], F32, tag="rstcat", space="PSUM")
                            gemm(lambda k: hT[:, k, w * P:w * P + mt], KCH, rhs_t, mt, pst)
                            ot = sb.tile([P, D], BF16, tag="gout")
                            nc.scalar.copy(out=ot[:mt, :], in_=pst[:mt, :D])
                            nc.sync.dma_start(out=dest[w * P:w * P + mt, :], in_=ot[:mt, :])
                    nc.gpsimd.collective_compute(
                        "AllGather", OP.bypass,
                        replica_groups=[list(range(NCORE))],
                        ins=[ag_in.opt()], outs=[fs_full_l[l].opt()],
                    )
                table = fs_full0 if l == 0 else fs_full_l[l]
                res_src = res_dram if l == 0 else h_dram

                # ================= edge phase =================
                cur_w = -1
                rst_ps = None
                calls = L["calls"]
                for ci, (wi, hf, koff, gcs) in enumerate(calls):
                    if wi != cur_w:
                        cur_w = wi
                        rst_ps = ps.tile([P, 264], F32, tag="rstcat", space="PSUM")
                        fdw = sb2.tile([P, D], BF16, tag="fdw")
                        nc.sync.dma_start(out=fdw[:], in_=fd_dram[wi * P:wi * P + P, :])
                        first_mm = True
                    # gather fs rows for up to 8 chunks per call
                    z8 = sb6.tile([P, 8, D], BF16, tag="z")
                    tab = table[:, :] if hf == 0 else table[HALF:, :]
                    nc.gpsimd.dma_gather(
                        z8[:, :gcs, :], tab, idx_t[:, koff * 8:koff * 8 + gcs * 8],
                        gcs * P, gcs * P, D, single_packet=False)
                    last_call = ci + 1 == len(calls) or calls[ci + 1][0] != wi
                    for sub in range(0, gcs, 4):
                        gs = min(4, gcs - sub)
                        ko = koff + sub
                        z = z8[:, sub:sub + 4, :]
                        # one-hot (edges on partitions)
                        oh = sb.tile([P, 4, P], BF16, tag="oh")
                        nc.vector.tensor_tensor(
                            out=oh[:, :gs, :],
                            in0=dstw_t[:, ko:ko + gs].to_broadcast([P, gs, P]),
                            in1=iota_t[:].rearrange("p (g j) -> p g j", g=4)[:, :gs, :],
                            op=OP.is_equal)
                        # transposed one-hot (dst on partitions) via PE
                        ohT_ps = ps.tile([P, 4 * P], BF16, tag="ohT", space="PSUM")
                        for j in range(gs):
                            nc.tensor.transpose(out=ohT_ps[:, j * P:(j + 1) * P],
                                                in_=oh[:, j, :], identity=ident16[:])
                        ohT = sb.tile([P, 4 * P], BF16, tag="ohTs")
                        nc.scalar.copy(out=ohT[:, :gs * P], in_=ohT_ps[:, :gs * P])
                        # z_psum = OneHot_ve.T @ fdw + fs  (= fs[src]+fd[dst])
                        zps = ps.tile([P, 4, D], F32, tag="zps", space="PSUM")
                        for j in range(gs):
                            nc.tensor.matmul(zps[:, j, :], lhsT=ohT[:, j * P:(j + 1) * P],
                                             rhs=fdw[:], start=True, stop=False)
                            nc.tensor.matmul(zps[:, j, :], lhsT=ident16[:],
                                             rhs=z[:, j, :], start=False, stop=True)
                        # leaky-relu = max(z, 0.2z): ACT Copy(scale) + DVE max
                        # (keeps ACT on one LUT set: Copy/Relu/Exp -> no table reloads)
                        lrs = sb.tile([P, 4, D], BF16, tag="lrs")
                        nc.scalar.activation(lrs[:, :gs, :], zps[:, :gs, :], AF.Copy,
                                             scale=SLOPE)
                        lr = sb.tile([P, 4, D], BF16, tag="lr")
                        nc.vector.tensor_tensor(out=lr[:, :gs, :], in0=zps[:, :gs, :],
                                                in1=lrs[:, :gs, :], op=OP.max)
                        sm = sb.tile([P, 4, D], BF16, tag="sm")
                        nc.vector.tensor_tensor(
                            out=sm[:, :gs, :], in0=lr[:, :gs, :],
                            in1=attn_t[l][:].rearrange("p (g d) -> p g d", g=4)[:, :gs, :],
                            op=OP.mult)
                        sc = sb.tile([P, 4, H], F32, tag="sc")
                        nc.vector.tensor_reduce(
                            out=sc[:, :gs, :],
                            in_=sm[:, :gs, :].rearrange("p g (h d) -> p g h d", h=H),
                            axis=AX.X, op=OP.add)
                        wcat = sb.tile([P, 4, 264], BF16, tag="wcat")
                        nc.scalar.activation(wcat[:, :gs, D:D + H], sc[:, :gs, :], AF.Exp)
                        # W = ex * z
                        nc.vector.tensor_tensor(
                            out=wcat[:, :gs, :D].rearrange("p g (h d) -> p g h d", h=H),
                            in0=zps[:, :gs, :].rearrange("p g (h d) -> p g h d", h=H),
                            in1=wcat[:, :gs, D:D + H].to_broadcast([P, gs, H, 32]),
                            op=OP.mult)
                        # accumulate [rstU | denom]
                        for j in range(gs):
                            last = last_call and sub + gs >= gcs and j == gs - 1
                            nc.tensor.matmul(rst_ps[:, :], lhsT=oh[:, j, :],
                                             rhs=wcat[:, j, :], start=first_mm, stop=last)
                            first_mm = False
                    # window epilogue
                    if ci + 1 == len(calls) or calls[ci + 1][0] != wi:
                        wt = min(P, SH - wi * P)
                        den = sb2.tile([P, H], F32, tag="den")
                        nc.vector.tensor_scalar_max(den[:], rst_ps[:, D:D + H], 1e-30)
                        rec = sb2.tile([P, H], F32, tag="rec")
                        nc.vector.reciprocal(rec[:], den[:])
                        msk = sb2.tile([P, H], F32, tag="msk")
                        nc.vector.tensor_scalar(out=msk[:], in0=rst_ps[:, D:D + H],
                                                scalar1=1e30, scalar2=1.0,
                                                op0=OP.mult, op1=OP.min)
                        rn = sb2.tile([P, D], F32, tag="rn")
                        nc.vector.tensor_tensor(
                            out=rn[:].rearrange("p (h d) -> p h d", h=H),
                            in0=rst_ps[:, :D].rearrange("p (h d) -> p h d", h=H),
                            in1=rec[:].to_broadcast([P, H, 32]), op=OP.mult)
                        fdw32 = sb2.tile([P, D], F32, tag="fdw32")
                        nc.vector.tensor_copy(out=fdw32[:], in_=fdw[:])
                        fdm = sb2.tile([P, D], F32, tag="fdm")
                        nc.vector.tensor_tensor(
                            out=fdm[:].rearrange("p (h d) -> p h d", h=H),
                            in0=fdw32[:].rearrange("p (h d) -> p h d", h=H),
                            in1=msk[:].to_broadcast([P, H, 32]), op=OP.mult)
                        nc.vector.tensor_tensor(out=rn[:], in0=rn[:], in1=fdm[:],
                                                op=OP.subtract)
                        rt = sb2.tile([P, D], F32, tag="rt")
                        nc.sync.dma_start(out=rt[:], in_=res_src[wi * P:wi * P + P, :])
                        nc.vector.tensor_tensor(out=rn[:], in0=rn[:], in1=rt[:], op=OP.add)
                        hsb = sb2.tile([P, D], F32, tag="hsb")
                        if act_relu:
                            nc.scalar.activation(hsb[:], rn[:], AF.Relu)
                        else:
                            nc.vector.tensor_copy(out=hsb[:], in_=rn[:])
                        if l < 2:
                            nc.sync.dma_start(out=h_dram[wi * P:wi * P + wt, :],
                                              in_=hsb[:wt, :])
                            for half in range(2):
                                tp = ps.tile([P, 4 * P], F32, tag="ohT", space="PSUM")
                                nc.tensor.transpose(out=tp[:, :P],
                                                    in_=hsb[:, half * P:(half + 1) * P],
                                                    identity=ident[:])
                                nc.scalar.copy(out=hT[:, half, wi * P:(wi + 1) * P],
                                               in_=tp[:, :P])
                        else:
                            mean = sb2.tile([P, 32], F32, tag="mean")
                            nc.vector.tensor_reduce(
                                out=mean[:],
                                in_=hsb[:].rearrange("p (h d) -> p d h", h=H),
                                axis=AX.X, op=OP.add)
                            osb = sb2.tile([P, 32], F32, tag="osb")
                            nc.scalar.mul(osb[:], mean[:], 1.0 / H)
                            nc.sync.dma_start(out=out_ext[wi * P:wi * P + wt, :],
                                              in_=osb[:wt, :])
    nc.compile()
    return nc


# ---------------------------------------------------------------- host driver
def prep_inputs(features, src, dst, Wsrc1, Wdst1, attn1, Wres1,
                Wsrc2, Wdst2, attn2, Wsrc3, Wdst3, attn3):
    N, IN = features.shape
    L = build_layout(np.asarray(src), np.asarray(dst), N)
    featT = np.ascontiguousarray(np.asarray(features).T)
    SH = L["SH"]

    def attn_rep(a):
        flat = np.asarray(a).reshape(-1)  # [256]
        return np.tile(np.tile(flat, 4)[None, :], (P, 1)).astype(np.float32)

    iota = np.tile(np.arange(P, dtype=np.float32)[None, :], (P, 4))
    ident = np.eye(P, dtype=np.float32)
    common = {
        "featT": featT, "ident": ident, "iota4": iota,
        "Wsrc0": np.asarray(Wsrc1), "Wdst0": np.asarray(Wdst1), "Wres1": np.asarray(Wres1),
        "Wsrc1": np.asarray(Wsrc2), "Wdst1": np.asarray(Wdst2),
        "Wsrc2": np.asarray(Wsrc3), "Wdst2": np.asarray(Wdst3),
        "attn4_0": attn_rep(attn1), "attn4_1": attn_rep(attn2), "attn4_2": attn_rep(attn3),
    }
    in_maps = []
    for c in range(NCORE):
        m = dict(common)
        m["featT_loc"] = np.ascontiguousarray(featT[:, c * SH:(c + 1) * SH])
        m["idx_w"] = L["idx_w"][c]
        m["dstw"] = L["dstw_cols"][c]
        in_maps.append(m)
    return L, in_maps


_BUILD_CACHE = {}


def run(features, src, dst, Wsrc1, Wdst1, attn1, Wres1,
        Wsrc2, Wdst2, attn2, Wsrc3, Wdst3, attn3, trace=False):
    N, IN = features.shape
    L, in_maps = prep_inputs(features, src, dst, Wsrc1, Wdst1, attn1, Wres1,
                             Wsrc2, Wdst2, attn2, Wsrc3, Wdst3, attn3)
    key = (N, IN, L["NCH"])
    if key not in _BUILD_CACHE:
        _BUILD_CACHE[key] = build_kernel(N, IN, L)
    nc = _BUILD_CACHE[key]
    res = run_bass_kernel_spmd(nc, in_maps, list(range(NCORE)), trace=trace,
                               trace_cores=list(range(NCORE)) if trace else None)
    out = np.concatenate([res.results[c]["out"] for c in range(NCORE)], axis=0)
    return out, res


def kernel(features, src, dst,
           Wsrc1, Wdst1, attn1, b1, Wres1,
           Wsrc2, Wdst2, attn2, b2,
           Wsrc3, Wdst3, attn3, b3):
    """Full-input entry point. Biases are zeros in this model (asserted)."""
    for b in (b1, b2, b3):
        assert float(np.abs(np.asarray(b)).max()) == 0.0, "nonzero bias unsupported"
    out, _ = run(np.asarray(features, np.float32), np.asarray(src), np.asarray(dst),
                 Wsrc1, Wdst1, attn1, Wres1, Wsrc2, Wdst2, attn2,
                 Wsrc3, Wdst3, attn3)
    return out.astype(np.float32)



# revision 28
# speedup vs baseline: 1.4351x; 1.4351x over previous
"""GATv2 (3-layer, 8-head) on 8 Trainium2 NeuronCores.

Strategy (edge-parallel, dst-sharded):
- Core c owns destination nodes [c*N/8, (c+1)*N/8) and all edges into them.
- The per-layer fs table (all N nodes) is split into two SEGMENT tensors:
  T1 = rows [0, R1) of every core's shard, T2 = rows [R1, SH).  Each segment
  is AllGather'd separately (single collective per Shared tensor), and each
  has < 32768 rows so gather indices are plain int16.
- Host sorts each core's edges by (segment, dst-window), pads to 128-edge
  chunks with a chunk structure uniform across cores (SPMD: one program).
- Edge phase runs in two passes: pass A (seg-0 edges) accumulates partial
  [rstU|den] per window and spills to DRAM; pass B (seg-1) reloads the
  partial, finishes the window, and runs the epilogue.  Pass A only needs
  T1, so T2's AllGather hides under it.
- Next-layer fs GEMMs are interleaved into pass-B epilogues (flush groups of
  4 windows; T1/T2 collectives issued at the segment boundaries), and the
  next-layer fd GEMMs run after the edge phase, under the T2 collective.
- Edge math per 128-dst window: dma_gather fs[src] rows, expand fd[dst] via
  one-hot matmul, zps = fs[src]+fd[dst] in PSUM, lr = Prelu(zps) on ACT,
  score = reduce(lr*attn) on DVE, ex = exp(score), W = ex*fs[src] (bf16
  SBUF), rstU = OneHot @ [W | ex] on TensorE, rst = rstU/den + residual.
- Output: mean over heads, host concatenates the 8 dst shards.
"""
import sys
sys.path.insert(0, "/opt/trn_rl_repo")
import numpy as np
import concourse.bass as bass
import concourse.mybir as mybir
import concourse.tile as tile
from concourse import bacc
from concourse.bass_utils import run_bass_kernel_spmd

P = 128
NCORE = 8
SLOPE = 0.2
H = 8

F32 = mybir.dt.float32
R32 = mybir.dt.float32r
BF16 = mybir.dt.bfloat16
I16 = mybir.dt.int16
AX = mybir.AxisListType
OP = mybir.AluOpType
AF = mybir.ActivationFunctionType


def seg_split(SH, NW):
    """Segment boundary R1 (rows): window-aligned, both segments < 32768/8."""
    if NW >= 2:
        R1 = (NW // 2) * P
        R1 = max(R1, SH - 32768 // NCORE)
        R1 = min(R1, 32768 // NCORE)
    else:
        R1 = SH // 2
    return R1


# ---------------------------------------------------------------- host layout
def build_layout(src, dst, N):
    """Edge layout: per-core, segment-major then dst-window-sorted, 128-padded,
    chunk structure uniform across cores."""
    SH = N // NCORE
    NW = (SH + P - 1) // P
    R1 = seg_split(SH, NW)
    cores = []
    for c in range(NCORE):
        m = (dst // SH) == c
        s, d = src[m], dst[m]
        dl = d - c * SH
        w = dl // P
        # source-node table mapping: segment + row within segment table
        sc = s // SH
        srl = s % SH
        sg = (srl >= R1).astype(np.int64)
        trow = np.where(sg == 0, sc * R1 + srl, sc * (SH - R1) + (srl - R1))
        order = np.lexsort((w, sg))
        s_t, dl, w, sg = trow[order], dl[order], w[order], sg[order]
        groups = {}
        for wi in range(NW):
            for g in range(2):
                gm = (w == wi) & (sg == g)
                groups[(wi, g)] = (s_t[gm], dl[gm])
        cores.append(groups)
    # uniform chunk counts per (window, segment)
    C = {}
    for wi in range(NW):
        for g in range(2):
            n = max(len(cores[c][(wi, g)][0]) for c in range(NCORE))
            C[(wi, g)] = (n + P - 1) // P
    NCH = sum(C.values())
    # flat edge arrays per core (memory order: window-major, segment inner)
    src_rel = np.zeros((NCORE, NCH * P), np.int16)
    dstw = np.full((NCORE, NCH * P), -1.0, np.float32)
    calls = []  # (window, segment, chunk_off, n_chunks)
    off = 0
    for wi in range(NW):
        for g in range(2):
            nch = C[(wi, g)]
            k = 0
            while k < nch:
                gs = min(8, nch - k)
                calls.append((wi, g, off + k, gs))
                k += gs
            for c in range(NCORE):
                s_t, dl = cores[c][(wi, g)]
                k0 = off * P
                src_rel[c, k0:k0 + len(s_t)] = s_t.astype(np.int16)
                dstw[c, k0:k0 + len(dl)] = (dl % P).astype(np.float32)
            off += nch
    assert off == NCH
    # segment-major call order: pass A (seg 0), then pass B (seg 1)
    calls = ([c for c in calls if c[1] == 0] + [c for c in calls if c[1] == 1])
    # wrapped int16 idx for dma_gather: per call, idx i -> [i%16, col+i//16]
    TOTC = NCH * P // 16
    idx_w = np.zeros((NCORE, P, TOTC), np.int16)
    for c in range(NCORE):
        w16 = src_rel[c].reshape(-1, 16).T  # [16, NCH*8]
        idx_w[c] = np.tile(w16, (8, 1))
    # chunk-column-major dstw: [P, NCH], edge k*P+p -> [p, k]
    dstw_cols = dstw.reshape(NCORE, NCH, P).transpose(0, 2, 1).copy()
    return dict(SH=SH, NW=NW, R1=R1, NCH=NCH, C=C, calls=calls,
                idx_w=idx_w, dstw_cols=dstw_cols)


# ---------------------------------------------------------------- bass kernel
def build_kernel(N, IN, L):
    """L = layout dict. IN = input feature dim (128)."""
    D = 256  # H*HID = H*OUT
    SH, NW, NCH, R1 = L["SH"], L["NW"], L["NCH"], L["R1"]
    SHP = NW * P                      # padded shard rows
    TOTC = NCH * P // 16
    S0, S1 = R1, SH - R1              # segment sizes (per core)

    nc = bacc.Bacc("TRN2", target_bir_lowering=False, debug=False,
                   num_devices=NCORE)
    # ---- inputs (per-core where noted)
    featT_loc = nc.declare_dram_parameter("featT_loc", [IN, SH], F32, isOutput=False)
    Ws = [nc.declare_dram_parameter(f"Wsrc{l}", [IN if l == 0 else D, D], F32, isOutput=False) for l in range(3)]
    Wd = [nc.declare_dram_parameter(f"Wdst{l}", [IN if l == 0 else D, D], F32, isOutput=False) for l in range(3)]
    Wres1 = nc.declare_dram_parameter("Wres1", [IN, D], F32, isOutput=False)
    attn4 = [nc.declare_dram_parameter(f"attn4_{l}", [P, 4 * D], F32, isOutput=False) for l in range(3)]
    iota4 = nc.declare_dram_parameter("iota4", [P, 4 * P], F32, isOutput=False)
    ident_in = nc.declare_dram_parameter("ident", [P, P], F32, isOutput=False)
    idx_in = nc.declare_dram_parameter("idx_w", [P, TOTC], I16, isOutput=False)   # per-core
    dstw_in = nc.declare_dram_parameter("dstw", [P, NCH], F32, isOutput=False)    # per-core
    out_ext = nc.declare_dram_parameter("out", [SH, 32], F32, isOutput=True)

    with tile.TileContext(nc) as tc:
        with (
            tc.tile_pool(name="const", bufs=1) as cpool,
            tc.tile_pool(name="sbuf", bufs=4) as sb,
            tc.tile_pool(name="stage", bufs=2) as sbst,
            tc.tile_pool(name="sb6", bufs=5) as sb6,
            tc.tile_pool(name="sb2", bufs=3) as sb2,
            tc.tile_pool(name="psum", bufs=2, space="PSUM") as ps,
            tc.tile_pool(name="dram", bufs=1, space="DRAM") as dr,
        ):
            # ---- persistent constants
            ident = cpool.tile([P, P], F32)
            nc.sync.dma_start(out=ident[:], in_=ident_in[:, :])
            ident16 = cpool.tile([P, P], BF16, tag="ident16")
            nc.vector.tensor_copy(out=ident16[:], in_=ident[:])
            iota_t = cpool.tile([P, 4 * P], F32)
            nc.sync.dma_start(out=iota_t[:], in_=iota4[:, :])
            idx_t = cpool.tile([P, TOTC], I16)
            nc.sync.dma_start(out=idx_t[:], in_=idx_in[:, :])
            dstw_t = cpool.tile([P, NCH], F32)
            nc.sync.dma_start(out=dstw_t[:], in_=dstw_in[:, :])
            attn_t = []
            for l in range(3):
                a32 = cpool.tile([P, 4 * D], F32, tag=f"attn32_{l}")
                nc.sync.dma_start(out=a32[:], in_=attn4[l][:, :])
                a = cpool.tile([P, 4 * D], BF16, tag=f"attn{l}")
                nc.vector.tensor_copy(out=a[:], in_=a32[:])
                attn_t.append(a)
            hT = cpool.tile([P, 2, SHP], BF16, tag="hT")  # local shard, transposed

            # ---- DRAM internals
            # per-layer segment tables (AllGather destinations)
            tseg = [[dr.tile([NCORE * S0, D], BF16, tag=f"t1_{l}", name=f"t1_{l}",
                             addr_space="Shared"),
                     dr.tile([NCORE * S1, D], BF16, tag=f"t2_{l}", name=f"t2_{l}",
                             addr_space="Shared")] for l in range(3)]
            ag_inA = dr.tile([R1, D], BF16, tag="aginA")
            ag_inB = dr.tile([SH - R1, D], BF16, tag="aginB")
            fd_bufs = [dr.tile([SHP, D], BF16, tag=f"fd{i}", name=f"fd{i}")
                       for i in range(2)]
            res_dram = dr.tile([SHP, D], F32, tag="res")
            h_dram = dr.tile([SHP, D], F32, tag="h")
            part_dram = dr.tile([SHP, 264], F32, tag="part")

            zero_sb = cpool.tile([P, D], F32, tag="zero")
            nc.vector.memset(zero_sb[:], 0.0)
            zero16 = cpool.tile([P, D], BF16, tag="zero16")
            nc.vector.memset(zero16[:], 0.0)
            if SHP > SH:  # zero the padded tails once
                for fdb in fd_bufs:
                    nc.sync.dma_start(out=fdb[SH:SHP, :], in_=zero16[:SHP - SH, :])
                nc.sync.dma_start(out=res_dram[SH:SHP, :], in_=zero_sb[:SHP - SH, :])
                nc.sync.dma_start(out=h_dram[SH:SHP, :], in_=zero_sb[:SHP - SH, :])

            def gemm(lhsT_ap_fn, kchunks, rhs_t, mt, out_psum):
                """out_psum[mt, D] = sum_k lhsT_k.T @ rhs_k (bf16 fast path)"""
                for k in range(kchunks):
                    nc.tensor.matmul(out_psum[:mt, :D],
                                     lhsT=lhsT_ap_fn(k),
                                     rhs=rhs_t[:, k, :],
                                     start=(k == 0), stop=(k == kchunks - 1))

            def load_w(wparam, kchunks, tag):
                w32 = sbst.tile([P, kchunks, D], F32, tag="wload", name="w32")
                nc.sync.dma_start(
                    out=w32[:], in_=wparam.ap().rearrange("(c k) n -> k c n", k=P))
                wt = cpool.tile([P, kchunks, D], BF16, tag=tag, name=f"w_{tag}")
                nc.vector.tensor_copy(out=wt[:], in_=w32[:])
                return wt

            wsrc_t = [load_w(Ws[l], 1 if l == 0 else 2, f"wsrc{l}") for l in range(3)]
            wdst_t = [load_w(Wd[l], 1 if l == 0 else 2, f"wdst{l}") for l in range(3)]
            wres_t = load_w(Wres1, 1, "wres")

            def ag_store(r0, rows, src_ap):
                """Store staged GEMM rows [r0, r0+rows) into the right
                per-segment ag buffer (never crosses the R1 boundary)."""
                if r0 < R1:
                    dst_ap = ag_inA[r0:r0 + rows, :]
                else:
                    dst_ap = ag_inB[r0 - R1:r0 - R1 + rows, :]
                if rows % P == 0:
                    dst_ap = dst_ap.rearrange("(c p) d -> p c d", p=P)
                nc.scalar.dma_start(out=dst_ap, in_=src_ap)

            def flush_and_gather(l, wi, wt, ag_st):
                """Flush the 4-window staging group ending at window wi;
                issue segment collectives at the segment boundaries."""
                w0 = wi - wi % 4
                nwin = wi - w0 + 1
                r0 = w0 * P
                if NW < 2:  # tiny: single window straddles R1
                    ag_store(0, R1, ag_st[:R1, 0, :])
                    ag_store(R1, SH - R1, ag_st[R1:SH, 0, :])
                elif wt == P:
                    ag_store(r0, nwin * P, ag_st[:, :nwin, :])
                else:
                    if nwin > 1:
                        ag_store(r0, (nwin - 1) * P, ag_st[:, :nwin - 1, :])
                    ag_store(wi * P, wt, ag_st[:wt, nwin - 1, :])
                r1c = min((wi + 1) * P, SH)
                if r1c == R1:  # segment-0 rows complete -> T1 collective
                    nc.gpsimd.collective_compute(
                        "AllGather", OP.bypass,
                        replica_groups=[list(range(NCORE))],
                        ins=[ag_inA.opt()], outs=[tseg[l][0].opt()])
                if r1c == SH:  # all rows complete -> T2 collective
                    nc.gpsimd.collective_compute(
                        "AllGather", OP.bypass,
                        replica_groups=[list(range(NCORE))],
                        ins=[ag_inB.opt()], outs=[tseg[l][1].opt()])
                    if NW < 2:  # degenerate tiny case: T1 never hit above
                        nc.gpsimd.collective_compute(
                            "AllGather", OP.bypass,
                            replica_groups=[list(range(NCORE))],
                            ins=[ag_inA.opt()], outs=[tseg[l][0].opt()])

            GF = 8  # tiles per batched load/store group

            def fd_pass(l, dest, res_too=False):
                """fd (and layer-0 res) GEMMs from resident hT / featT_loc."""
                for g0 in range(0, NW, GF):
                    gn = min(GF, NW - g0)
                    if l == 0:
                        ft32 = sbst.tile([P, GF * P], F32, tag="ft")
                        cw = min(GF * P, SH - g0 * P)
                        nc.sync.dma_start(out=ft32[:, :cw],
                                          in_=featT_loc[:, g0 * P:g0 * P + cw])
                        ft = sbst.tile([P, GF * P], BF16, tag="ftb")
                        nc.scalar.copy(out=ft[:, :cw], in_=ft32[:, :cw])
                    dests = [(wdst_t[l], dest, BF16)]
                    if res_too:
                        dests.append((wres_t, res_dram, F32))
                    for (rhs_t, dst_dram, dt_) in dests:
                        st = sbst.tile([P, GF, D], dt_,
                                       tag="fsst" if dt_ is BF16 else "fsst32")
                        for j in range(gn):
                            w = g0 + j
                            mt = min(P, SH - w * P)
                            pst = ps.tile([P, 264], F32, tag="rstcat", space="PSUM")
                            if l == 0:
                                nc.tensor.matmul(
                                    pst[:mt, :D],
                                    lhsT=ft[:, j * P:j * P + mt],
                                    rhs=rhs_t[:, 0, :],
                                    start=True, stop=True)
                            else:
                                gemm(lambda k: hT[:, k, w * P:w * P + mt], 2,
                                     rhs_t, mt, pst)
                            nc.scalar.copy(out=st[:mt, j, :], in_=pst[:mt, :D])
                        rows = min(GF * P, SH - g0 * P)
                        nfull = rows // P
                        r0 = g0 * P
                        if nfull:
                            nc.scalar.dma_start(
                                out=dst_dram[r0:r0 + nfull * P, :].rearrange(
                                    "(c p) d -> p c d", p=P),
                                in_=st[:, :nfull, :])
                        if rows % P:
                            nc.scalar.dma_start(
                                out=dst_dram[r0 + nfull * P:r0 + rows, :],
                                in_=st[:rows % P, nfull, :])

            # ================= layer 0 dense: sharded fs GEMM + collectives
            ag_st = None
            ft0 = None
            for w in range(NW):
                wt = min(P, SH - w * P)
                if w % GF == 0:
                    ft32 = sbst.tile([P, GF * P], F32, tag="ft")
                    cw = min(GF * P, SH - w * P)
                    nc.sync.dma_start(out=ft32[:, :cw],
                                      in_=featT_loc[:, w * P:w * P + cw])
                    ft0 = sbst.tile([P, GF * P], BF16, tag="ftb")
                    nc.scalar.copy(out=ft0[:, :cw], in_=ft32[:, :cw])
                if w % 4 == 0:
                    ag_st = sb2.tile([P, 4, D], BF16, tag="agst")
                pst = ps.tile([P, 264], F32, tag="rstcat", space="PSUM")
                nc.tensor.matmul(pst[:wt, :D],
                                 lhsT=ft0[:, (w % GF) * P:(w % GF) * P + wt],
                                 rhs=wsrc_t[0][:, 0, :],
                                 start=True, stop=True)
                nc.scalar.copy(out=ag_st[:wt, w % 4, :], in_=pst[:wt, :D])
                if w % 4 == 3 or w == NW - 1:
                    flush_and_gather(0, w, wt, ag_st)
            fd_pass(0, fd_bufs[0], res_too=True)

            calls = L["calls"]
            nA = sum(1 for c in calls if c[1] == 0)

            for l in range(3):
                act_relu = l < 2
                res_src = res_dram if l == 0 else h_dram
                fd_cur = fd_bufs[l % 2]
                ag_st = None

                # ================= edge phase: pass A (seg 0), pass B (seg 1)
                cur_w = -1
                cur_sg = -1
                rst_ps = None
                for ci, (wi, sg, koff, gcs) in enumerate(calls):
                    if wi != cur_w or sg != cur_sg:
                        cur_w, cur_sg = wi, sg
                        rst_ps = ps.tile([P, 264], F32, tag="rstcat", space="PSUM")
                        fdw = sb2.tile([P, D], BF16, tag="fdw")
                        nc.sync.dma_start(out=fdw[:], in_=fd_cur[wi * P:wi * P + P, :])
                        if sg == 1:
                            # reload pass-A partial and seed the accumulator
                            prt = sb2.tile([P, 264], F32, tag="prt")
                            nc.sync.dma_start(out=prt[:],
                                              in_=part_dram[wi * P:(wi + 1) * P, :])
                            nc.tensor.matmul(rst_ps[:, :],
                                             lhsT=ident[:],
                                             rhs=prt[:],
                                             start=True, stop=False)
                            first_mm = False
                        else:
                            first_mm = True
                    # gather fs rows for up to 8 chunks per call
                    z8 = sb6.tile([P, 8, D], BF16, tag="z")
                    tab = tseg[l][sg]
                    nc.gpsimd.dma_gather(
                        z8[:, :gcs, :], tab[:, :], idx_t[:, koff * 8:koff * 8 + gcs * 8],
                        gcs * P, gcs * P, D, single_packet=False)
                    last_call = (ci + 1 == len(calls) or calls[ci + 1][0] != wi
                                 or calls[ci + 1][1] != sg)
                    for sub in range(0, gcs, 4):
                        gs = min(4, gcs - sub)
                        ko = koff + sub
                        z = z8[:, sub:sub + 4, :]
                        # one-hot (edges on partitions)
                        oh = sb.tile([P, 4, P], BF16, tag="oh")
                        nc.vector.tensor_tensor(
                            out=oh[:, :gs, :],
                            in0=dstw_t[:, ko:ko + gs].to_broadcast([P, gs, P]),
                            in1=iota_t[:].rearrange("p (g j) -> p g j", g=4)[:, :gs, :],
                            op=OP.is_equal)
                        # transposed one-hot (dst on partitions) via PE
                        ohT_ps = ps.tile([P, 4 * P], BF16, tag="ohT", space="PSUM")
                        for j in range(gs):
                            nc.tensor.transpose(out=ohT_ps[:, j * P:(j + 1) * P],
                                                in_=oh[:, j, :], identity=ident16[:])
                        ohT = sb.tile([P, 4 * P], BF16, tag="ohTs")
                        nc.scalar.copy(out=ohT[:, :gs * P], in_=ohT_ps[:, :gs * P])
                        # z_psum = OneHot_ve.T @ fdw + fs  (= fs[src]+fd[dst])
                        zps = ps.tile([P, 4, D], F32, tag="zps", space="PSUM")
                        for j in range(gs):
                            nc.tensor.matmul(zps[:, j, :], lhsT=ohT[:, j * P:(j + 1) * P],
                                             rhs=fdw[:], start=True, stop=False)
                            nc.tensor.matmul(zps[:, j, :], lhsT=ident16[:],
                                             rhs=z[:, j, :], start=False, stop=True)
                        # leaky-relu in one ACT pass: Prelu (parametric relu,
                        # same LUT set as Copy/Relu/Exp -> no table reloads)
                        lr = sb.tile([P, 4, D], BF16, tag="lr")
                        nc.scalar.activation(lr[:, :gs, :], zps[:, :gs, :], AF.Prelu,
                                             alpha=SLOPE)
                        sm = sb.tile([P, 4, D], BF16, tag="sm")
                        nc.vector.tensor_tensor(
                            out=sm[:, :gs, :], in0=lr[:, :gs, :],
                            in1=attn_t[l][:].rearrange("p (g d) -> p g d", g=4)[:, :gs, :],
                            op=OP.mult)
                        sc = sb.tile([P, 4, H], F32, tag="sc")
                        nc.vector.tensor_reduce(
                            out=sc[:, :gs, :],
                            in_=sm[:, :gs, :].rearrange("p g (d h) -> p g h d", h=H),
                            axis=AX.X, op=OP.add)
                        wcat = sb.tile([P, 4, 264], BF16, tag="wcat")
                        nc.scalar.activation(wcat[:, :gs, D:D + H], sc[:, :gs, :], AF.Exp)
                        # W = ex * fs[src] (z8, bf16 SBUF; d-major layout
                        # keeps every last dim packed -> DVE 2x mode)
                        nc.vector.tensor_tensor(
                            out=wcat[:, :gs, :D].rearrange("p g (d h) -> p g d h", h=H),
                            in0=z[:, :gs, :].rearrange("p g (d h) -> p g d h", h=H),
                            in1=wcat[:, :gs, D:D + H].unsqueeze(2).to_broadcast(
                                [P, gs, D // H, H]),
                            op=OP.mult)
                        # accumulate [rstU | denom]
                        for j in range(gs):
                            last = last_call and sub + gs >= gcs and j == gs - 1
                            nc.tensor.matmul(rst_ps[:, :], lhsT=oh[:, j, :],
                                             rhs=wcat[:, j, :], start=first_mm, stop=last)
                            first_mm = False
                    if not last_call:
                        continue
                    if sg == 0:
                        # pass A: spill partial [rstU|den] for this window
                        pt = sb2.tile([P, 264], F32, tag="pt")
                        nc.scalar.copy(out=pt[:], in_=rst_ps[:, :])
                        nc.scalar.dma_start(out=part_dram[wi * P:(wi + 1) * P, :],
                                            in_=pt[:])
                        continue
                    # ---- window epilogue (pass B)
                    wt = min(P, SH - wi * P)
                    den = sb2.tile([P, H], F32, tag="den")
                    nc.vector.tensor_scalar_max(den[:], rst_ps[:, D:D + H], 1e-30)
                    rec = sb2.tile([P, H], F32, tag="rec")
                    nc.vector.reciprocal(rec[:], den[:])
                    rn = sb2.tile([P, D], F32, tag="rn")
                    nc.vector.tensor_tensor(
                        out=rn[:].rearrange("p (d h) -> p d h", h=H),
                        in0=rst_ps[:, :D].rearrange("p (d h) -> p d h", h=H),
                        in1=rec[:].unsqueeze(1).to_broadcast([P, D // H, H]),
                        op=OP.mult)
                    rt = sb2.tile([P, D], F32, tag="rt")
                    nc.sync.dma_start(out=rt[:], in_=res_src[wi * P:wi * P + P, :])
                    nc.vector.tensor_tensor(out=rn[:], in0=rn[:], in1=rt[:], op=OP.add)
                    hsb = sb2.tile([P, D], F32, tag="hsb")
                    if act_relu:
                        nc.scalar.activation(hsb[:], rn[:], AF.Relu)
                    else:
                        nc.vector.tensor_copy(out=hsb[:], in_=rn[:])
                    if l < 2:
                        nc.scalar.dma_start(out=h_dram[wi * P:wi * P + wt, :],
                                            in_=hsb[:wt, :])
                        for half in range(2):
                            tp = ps.tile([P, 4 * P], F32, tag="ohT", space="PSUM")
                            nc.tensor.transpose(out=tp[:, :P],
                                                in_=hsb[:, half * P:(half + 1) * P],
                                                identity=ident[:])
                            nc.scalar.copy(out=hT[:, half, wi * P:(wi + 1) * P],
                                           in_=tp[:, :P])
                        # interleaved next-layer fs GEMM for this window
                        if wi % 4 == 0:
                            ag_st = sb2.tile([P, 4, D], BF16, tag="agst")
                        pst = ps.tile([P, 264], F32, tag="ohT", space="PSUM")
                        gemm(lambda k: hT[:, k, wi * P:wi * P + wt], 2,
                             wsrc_t[l + 1], wt, pst)
                        nc.scalar.copy(out=ag_st[:wt, wi % 4, :], in_=pst[:wt, :D])
                        if wi % 4 == 3 or wi == NW - 1:
                            flush_and_gather(l + 1, wi, wt, ag_st)
                    else:
                        mean = sb2.tile([P, 32], F32, tag="mean")
                        nc.vector.tensor_reduce(
                            out=mean[:],
                            in_=hsb[:].rearrange("p (d h) -> p d h", h=H),
                            axis=AX.X, op=OP.add)
                        osb = sb2.tile([P, 32], F32, tag="osb")
                        nc.scalar.mul(osb[:], mean[:], 1.0 / H)
                        nc.scalar.dma_start(out=out_ext[wi * P:wi * P + wt, :],
                                            in_=osb[:wt, :])

                # fd GEMMs for the next layer (overlap the T2 collective)
                if l < 2:
                    fd_pass(l + 1, fd_bufs[(l + 1) % 2])
    nc.compile()
    return nc


# ---------------------------------------------------------------- host driver
def prep_inputs(features, src, dst, Wsrc1, Wdst1, attn1, Wres1,
                Wsrc2, Wdst2, attn2, Wsrc3, Wdst3, attn3):
    N, IN = features.shape
    L = build_layout(np.asarray(src), np.asarray(dst), N)
    featT = np.ascontiguousarray(np.asarray(features).T)
    SH = L["SH"]

    # d-major column order: new col j = (d, h) with j = d*8+h
    perm = np.array([(j % H) * 32 + j // H for j in range(256)])

    def attn_rep(a):
        flat = np.asarray(a).T.reshape(-1)  # [256] d-major
        return np.tile(np.tile(flat, 4)[None, :], (P, 1)).astype(np.float32)

    iota = np.tile(np.arange(P, dtype=np.float32)[None, :], (P, 4))
    ident = np.eye(P, dtype=np.float32)
    common = {
        "ident": ident, "iota4": iota,
        "Wsrc0": np.asarray(Wsrc1)[:, perm], "Wdst0": np.asarray(Wdst1)[:, perm],
        "Wres1": np.asarray(Wres1)[:, perm],
        "Wsrc1": np.asarray(Wsrc2)[perm][:, perm], "Wdst1": np.asarray(Wdst2)[perm][:, perm],
        "Wsrc2": np.asarray(Wsrc3)[perm][:, perm], "Wdst2": np.asarray(Wdst3)[perm][:, perm],
        "attn4_0": attn_rep(attn1), "attn4_1": attn_rep(attn2), "attn4_2": attn_rep(attn3),
    }
    in_maps = []
    for c in range(NCORE):
        m = dict(common)
        m["featT_loc"] = np.ascontiguousarray(featT[:, c * SH:(c + 1) * SH])
        m["idx_w"] = L["idx_w"][c]
        m["dstw"] = L["dstw_cols"][c]
        in_maps.append(m)
    return L, in_maps


_BUILD_CACHE = {}


def run(features, src, dst, Wsrc1, Wdst1, attn1, Wres1,
        Wsrc2, Wdst2, attn2, Wsrc3, Wdst3, attn3, trace=False):
    N, IN = features.shape
    L, in_maps = prep_inputs(features, src, dst, Wsrc1, Wdst1, attn1, Wres1,
                             Wsrc2, Wdst2, attn2, Wsrc3, Wdst3, attn3)
    key = (N, IN, L["NCH"])
    if key not in _BUILD_CACHE:
        _BUILD_CACHE[key] = build_kernel(N, IN, L)
    nc = _BUILD_CACHE[key]
    res = run_bass_kernel_spmd(nc, in_maps, list(range(NCORE)), trace=trace,
                               trace_cores=list(range(NCORE)) if trace else None)
    out = np.concatenate([res.results[c]["out"] for c in range(NCORE)], axis=0)
    return out, res


def kernel(features, src, dst,
           Wsrc1, Wdst1, attn1, b1, Wres1,
           Wsrc2, Wdst2, attn2, b2,
           Wsrc3, Wdst3, attn3, b3):
    """Full-input entry point. Biases are zeros in this model (asserted)."""
    for b in (b1, b2, b3):
        assert float(np.abs(np.asarray(b)).max()) == 0.0, "nonzero bias unsupported"
    out, _ = run(np.asarray(features, np.float32), np.asarray(src), np.asarray(dst),
                 Wsrc1, Wdst1, attn1, Wres1, Wsrc2, Wdst2, attn2,
                 Wsrc3, Wdst3, attn3)
    return out.astype(np.float32)


# revision 39
# speedup vs baseline: 1.6626x; 1.1586x over previous
"""GATv2 (3-layer, 8-head) on 8 Trainium2 NeuronCores.

Strategy (edge-parallel, dst-sharded):
- Core c owns destination nodes [c*N/8, (c+1)*N/8) and all edges into them.
- The per-layer fs table (all N nodes) is split into two SEGMENT tensors:
  T1 = rows [0, R1) of every core's shard, T2 = rows [R1, SH).  Each segment
  is AllGather'd separately (single collective per Shared tensor), and each
  has < 32768 rows so gather indices are plain int16.
- Host sorts each core's edges by (segment, dst-window), pads to 128-edge
  chunks with a chunk structure uniform across cores (SPMD: one program).
- Edge phase runs in two passes: pass A (seg-0 edges) accumulates partial
  [rstU|den] per window and spills to DRAM; pass B (seg-1) reloads the
  partial, finishes the window, and runs the epilogue.  Pass A only needs
  T1, so T2's AllGather hides under it.
- Next-layer fs GEMMs are interleaved into pass-B epilogues (flush groups of
  4 windows; T1/T2 collectives issued at the segment boundaries), and the
  next-layer fd GEMMs run after the edge phase, under the T2 collective.
- Edge math per 128-dst window: dma_gather fs[src] rows, expand fd[dst] via
  one-hot matmul, zps = fs[src]+fd[dst] in PSUM, lr = Prelu(zps) on ACT,
  score = reduce(lr*attn) on DVE, ex = exp(score), W = ex*fs[src] (bf16
  SBUF), rstU = OneHot @ [W | ex] on TensorE, rst = rstU/den + residual.
- Output: mean over heads, host concatenates the 8 dst shards.
"""
import sys
sys.path.insert(0, "/opt/trn_rl_repo")
import numpy as np
import concourse.bass as bass
import concourse.mybir as mybir
import concourse.tile as tile
from concourse import bacc
from concourse.bass_utils import run_bass_kernel_spmd

P = 128
NCORE = 8
SLOPE = 0.2
H = 8

F32 = mybir.dt.float32
R32 = mybir.dt.float32r
BF16 = mybir.dt.bfloat16
I16 = mybir.dt.int16
AX = mybir.AxisListType
OP = mybir.AluOpType
AF = mybir.ActivationFunctionType


def seg_split(SH, NW):
    """Segment boundary R1 (rows): window-aligned, both segments < 32768/8."""
    if NW >= 2:
        R1 = (NW // 2) * P
        R1 = max(R1, SH - 32768 // NCORE)
        R1 = min(R1, 32768 // NCORE)
    else:
        R1 = SH // 2
    return R1


# ---------------------------------------------------------------- host layout
def build_layout(src, dst, N):
    """Edge layout: per-core, segment-major then dst-window-sorted, 128-padded,
    chunk structure uniform across cores."""
    SH = N // NCORE
    NW = (SH + P - 1) // P
    R1 = seg_split(SH, NW)
    cores = []
    for c in range(NCORE):
        m = (dst // SH) == c
        s, d = src[m], dst[m]
        dl = d - c * SH
        w = dl // P
        # source-node table mapping: segment + row within segment table
        sc = s // SH
        srl = s % SH
        sg = (srl >= R1).astype(np.int64)
        trow = np.where(sg == 0, sc * R1 + srl, sc * (SH - R1) + (srl - R1))
        order = np.lexsort((w, sg))
        s_t, dl, w, sg = trow[order], dl[order], w[order], sg[order]
        groups = {}
        for wi in range(NW):
            for g in range(2):
                gm = (w == wi) & (sg == g)
                groups[(wi, g)] = (s_t[gm], dl[gm])
        cores.append(groups)
    # uniform chunk counts per (window, segment)
    C = {}
    for wi in range(NW):
        for g in range(2):
            n = max(len(cores[c][(wi, g)][0]) for c in range(NCORE))
            C[(wi, g)] = (n + P - 1) // P
    NCH = sum(C.values())
    # flat edge arrays per core (memory order: window-major, segment inner)
    src_rel = np.zeros((NCORE, NCH * P), np.int16)
    dstw = np.full((NCORE, NCH * P), -1.0, np.float32)
    calls = []  # (window, segment, chunk_off, n_chunks)
    off = 0
    for wi in range(NW):
        for g in range(2):
            nch = C[(wi, g)]
            k = 0
            while k < nch:
                gs = min(12, nch - k)
                calls.append((wi, g, off + k, gs))
                k += gs
            for c in range(NCORE):
                s_t, dl = cores[c][(wi, g)]
                k0 = off * P
                src_rel[c, k0:k0 + len(s_t)] = s_t.astype(np.int16)
                dstw[c, k0:k0 + len(dl)] = (dl % P).astype(np.float32)
            off += nch
    assert off == NCH
    # segment-major call order: pass A (seg 0), then pass B (seg 1)
    calls = ([c for c in calls if c[1] == 0] + [c for c in calls if c[1] == 1])
    # wrapped int16 idx for dma_gather: per call, idx i -> [i%16, col+i//16]
    TOTC = NCH * P // 16
    idx_w = np.zeros((NCORE, P, TOTC), np.int16)
    for c in range(NCORE):
        w16 = src_rel[c].reshape(-1, 16).T  # [16, NCH*8]
        idx_w[c] = np.tile(w16, (8, 1))
    # chunk-column-major dstw: [P, NCH], edge k*P+p -> [p, k]
    dstw_cols = dstw.reshape(NCORE, NCH, P).transpose(0, 2, 1).copy()
    return dict(SH=SH, NW=NW, R1=R1, NCH=NCH, C=C, calls=calls,
                idx_w=idx_w, dstw_cols=dstw_cols)


# ---------------------------------------------------------------- bass kernel
def build_kernel(N, IN, L):
    """L = layout dict. IN = input feature dim (128)."""
    D = 256  # H*HID = H*OUT
    SH, NW, NCH, R1 = L["SH"], L["NW"], L["NCH"], L["R1"]
    SHP = NW * P                      # padded shard rows
    TOTC = NCH * P // 16
    S0, S1 = R1, SH - R1              # segment sizes (per core)

    nc = bacc.Bacc("TRN2", target_bir_lowering=False, debug=False,
                   num_devices=NCORE)
    # ---- inputs (per-core where noted)
    featT_loc = nc.declare_dram_parameter("featT_loc", [IN, SH], F32, isOutput=False)
    Ws = [nc.declare_dram_parameter(f"Wsrc{l}", [IN if l == 0 else D, D], F32, isOutput=False) for l in range(3)]
    Wd = [nc.declare_dram_parameter(f"Wdst{l}", [IN if l == 0 else D, D], F32, isOutput=False) for l in range(3)]
    Wres1 = nc.declare_dram_parameter("Wres1", [IN, D], F32, isOutput=False)
    attn4 = [nc.declare_dram_parameter(f"attn4_{l}", [P, 4 * D], F32, isOutput=False) for l in range(3)]
    iota4 = nc.declare_dram_parameter("iota4", [P, 4 * P], F32, isOutput=False)
    ident_in = nc.declare_dram_parameter("ident", [P, P], F32, isOutput=False)
    idx_in = nc.declare_dram_parameter("idx_w", [P, TOTC], I16, isOutput=False)   # per-core
    dstw_in = nc.declare_dram_parameter("dstw", [P, NCH], F32, isOutput=False)    # per-core
    out_ext = nc.declare_dram_parameter("out", [SH, 32], F32, isOutput=True)

    with tile.TileContext(nc) as tc:
        with (
            tc.tile_pool(name="const", bufs=1) as cpool,
            tc.tile_pool(name="sbuf", bufs=4) as sb,
            tc.tile_pool(name="stage", bufs=2) as sbst,
            tc.tile_pool(name="sb6", bufs=5) as sb6,
            tc.tile_pool(name="sb2", bufs=3) as sb2,
            tc.tile_pool(name="psum", bufs=2, space="PSUM") as ps,
            tc.tile_pool(name="dram", bufs=1, space="DRAM") as dr,
        ):
            # ---- persistent constants
            ident = cpool.tile([P, P], F32)
            nc.sync.dma_start(out=ident[:], in_=ident_in[:, :])
            ident16 = cpool.tile([P, P], BF16, tag="ident16")
            nc.vector.tensor_copy(out=ident16[:], in_=ident[:])
            iota_t = cpool.tile([P, 4 * P], F32)
            nc.sync.dma_start(out=iota_t[:], in_=iota4[:, :])
            idx_t = cpool.tile([P, TOTC], I16)
            nc.sync.dma_start(out=idx_t[:], in_=idx_in[:, :])
            dstw_t = cpool.tile([P, NCH], F32)
            nc.sync.dma_start(out=dstw_t[:], in_=dstw_in[:, :])
            attn_t = []
            for l in range(3):
                a32 = cpool.tile([P, 4 * D], F32, tag=f"attn32_{l}")
                nc.sync.dma_start(out=a32[:], in_=attn4[l][:, :])
                a = cpool.tile([P, 4 * D], BF16, tag=f"attn{l}")
                nc.vector.tensor_copy(out=a[:], in_=a32[:])
                attn_t.append(a)
            hT = cpool.tile([P, 2, SHP], BF16, tag="hT")  # local shard, transposed

            # ---- DRAM internals
            # per-layer segment tables (AllGather destinations)
            tseg = [[dr.tile([NCORE * S0, D], BF16, tag=f"t1_{l}", name=f"t1_{l}",
                             addr_space="Shared"),
                     dr.tile([NCORE * S1, D], BF16, tag=f"t2_{l}", name=f"t2_{l}",
                             addr_space="Shared")] for l in range(3)]
            ag_inA = dr.tile([R1, D], BF16, tag="aginA")
            ag_inB = dr.tile([SH - R1, D], BF16, tag="aginB")
            fd_bufs = [dr.tile([SHP, D], BF16, tag=f"fd{i}", name=f"fd{i}")
                       for i in range(2)]
            res_dram = dr.tile([SHP, D], F32, tag="res")
            h_dram = dr.tile([SHP, D], F32, tag="h")
            part_dram = dr.tile([SHP, 264], F32, tag="part")

            for _zi in range(5):
                zz = sb6.tile([P, 12, D], BF16, tag="z", name="zinit")
                nc.vector.memset(zz[:], 0.0)
            zero_sb = cpool.tile([P, D], F32, tag="zero")
            nc.vector.memset(zero_sb[:], 0.0)
            zero16 = cpool.tile([P, D], BF16, tag="zero16")
            nc.vector.memset(zero16[:], 0.0)
            if SHP > SH:  # zero the padded tails once
                for fdb in fd_bufs:
                    nc.sync.dma_start(out=fdb[SH:SHP, :], in_=zero16[:SHP - SH, :])
                nc.sync.dma_start(out=res_dram[SH:SHP, :], in_=zero_sb[:SHP - SH, :])
                nc.sync.dma_start(out=h_dram[SH:SHP, :], in_=zero_sb[:SHP - SH, :])

            def gemm(lhsT_ap_fn, kchunks, rhs_t, mt, out_psum):
                """out_psum[mt, D] = sum_k lhsT_k.T @ rhs_k (bf16 fast path)"""
                for k in range(kchunks):
                    nc.tensor.matmul(out_psum[:mt, :D],
                                     lhsT=lhsT_ap_fn(k),
                                     rhs=rhs_t[:, k, :],
                                     start=(k == 0), stop=(k == kchunks - 1))

            def load_w(wparam, kchunks, tag):
                w32 = sbst.tile([P, kchunks, D], F32, tag="wload", name="w32")
                nc.sync.dma_start(
                    out=w32[:], in_=wparam.ap().rearrange("(c k) n -> k c n", k=P))
                wt = cpool.tile([P, kchunks, D], BF16, tag=tag, name=f"w_{tag}")
                nc.vector.tensor_copy(out=wt[:], in_=w32[:])
                return wt

            wsrc_t = [load_w(Ws[l], 1 if l == 0 else 2, f"wsrc{l}") for l in range(3)]
            wdst_t = [load_w(Wd[l], 1 if l == 0 else 2, f"wdst{l}") for l in range(3)]
            wres_t = load_w(Wres1, 1, "wres")

            def ag_store(r0, rows, src_ap):
                """Store staged GEMM rows [r0, r0+rows) into the right
                per-segment ag buffer (never crosses the R1 boundary)."""
                if r0 < R1:
                    dst_ap = ag_inA[r0:r0 + rows, :]
                else:
                    dst_ap = ag_inB[r0 - R1:r0 - R1 + rows, :]
                if rows % P == 0:
                    dst_ap = dst_ap.rearrange("(c p) d -> p c d", p=P)
                nc.scalar.dma_start(out=dst_ap, in_=src_ap)

            def flush_and_gather(l, wi, wt, ag_st):
                """Flush the 4-window staging group ending at window wi;
                issue segment collectives at the segment boundaries."""
                w0 = wi - wi % 4
                nwin = wi - w0 + 1
                r0 = w0 * P
                if NW < 2:  # tiny: single window straddles R1
                    ag_store(0, R1, ag_st[:R1, 0, :])
                    ag_store(R1, SH - R1, ag_st[R1:SH, 0, :])
                elif wt == P:
                    ag_store(r0, nwin * P, ag_st[:, :nwin, :])
                else:
                    if nwin > 1:
                        ag_store(r0, (nwin - 1) * P, ag_st[:, :nwin - 1, :])
                    ag_store(wi * P, wt, ag_st[:wt, nwin - 1, :])
                r1c = min((wi + 1) * P, SH)
                if r1c == R1:  # segment-0 rows complete -> T1 collective
                    nc.gpsimd.collective_compute(
                        "AllGather", OP.bypass,
                        replica_groups=[list(range(NCORE))],
                        ins=[ag_inA.opt()], outs=[tseg[l][0].opt()])
                if r1c == SH:  # all rows complete -> T2 collective
                    nc.gpsimd.collective_compute(
                        "AllGather", OP.bypass,
                        replica_groups=[list(range(NCORE))],
                        ins=[ag_inB.opt()], outs=[tseg[l][1].opt()])
                    if NW < 2:  # degenerate tiny case: T1 never hit above
                        nc.gpsimd.collective_compute(
                            "AllGather", OP.bypass,
                            replica_groups=[list(range(NCORE))],
                            ins=[ag_inA.opt()], outs=[tseg[l][0].opt()])

            GF = 8  # tiles per batched load/store group

            def fd_pass(l, dest, res_too=False):
                """fd (and layer-0 res) GEMMs from resident hT / featT_loc."""
                for g0 in range(0, NW, GF):
                    gn = min(GF, NW - g0)
                    if l == 0:
                        ft32 = sbst.tile([P, GF * P], F32, tag="ft")
                        cw = min(GF * P, SH - g0 * P)
                        nc.sync.dma_start(out=ft32[:, :cw],
                                          in_=featT_loc[:, g0 * P:g0 * P + cw])
                        ft = sbst.tile([P, GF * P], BF16, tag="ftb")
                        nc.scalar.copy(out=ft[:, :cw], in_=ft32[:, :cw])
                    dests = [(wdst_t[l], dest, BF16)]
                    if res_too:
                        dests.append((wres_t, res_dram, F32))
                    for (rhs_t, dst_dram, dt_) in dests:
                        st = sbst.tile([P, GF, D], dt_,
                                       tag="fsst" if dt_ is BF16 else "fsst32")
                        for j in range(gn):
                            w = g0 + j
                            mt = min(P, SH - w * P)
                            pst = ps.tile([P, 264], F32, tag="rstcat", space="PSUM")
                            if l == 0:
                                nc.tensor.matmul(
                                    pst[:mt, :D],
                                    lhsT=ft[:, j * P:j * P + mt],
                                    rhs=rhs_t[:, 0, :],
                                    start=True, stop=True)
                            else:
                                gemm(lambda k: hT[:, k, w * P:w * P + mt], 2,
                                     rhs_t, mt, pst)
                            nc.scalar.copy(out=st[:mt, j, :], in_=pst[:mt, :D])
                        rows = min(GF * P, SH - g0 * P)
                        nfull = rows // P
                        r0 = g0 * P
                        if nfull:
                            nc.scalar.dma_start(
                                out=dst_dram[r0:r0 + nfull * P, :].rearrange(
                                    "(c p) d -> p c d", p=P),
                                in_=st[:, :nfull, :])
                        if rows % P:
                            nc.scalar.dma_start(
                                out=dst_dram[r0 + nfull * P:r0 + rows, :],
                                in_=st[:rows % P, nfull, :])

            # ================= layer 0 dense: sharded fs GEMM + collectives
            ag_st = None
            ft0 = None
            for w in range(NW):
                wt = min(P, SH - w * P)
                if w % GF == 0:
                    ft32 = sbst.tile([P, GF * P], F32, tag="ft")
                    cw = min(GF * P, SH - w * P)
                    nc.sync.dma_start(out=ft32[:, :cw],
                                      in_=featT_loc[:, w * P:w * P + cw])
                    ft0 = sbst.tile([P, GF * P], BF16, tag="ftb")
                    nc.scalar.copy(out=ft0[:, :cw], in_=ft32[:, :cw])
                if w % 4 == 0:
                    ag_st = sb2.tile([P, 4, D], BF16, tag="agst")
                pst = ps.tile([P, 264], F32, tag="rstcat", space="PSUM")
                nc.tensor.matmul(pst[:wt, :D],
                                 lhsT=ft0[:, (w % GF) * P:(w % GF) * P + wt],
                                 rhs=wsrc_t[0][:, 0, :],
                                 start=True, stop=True)
                nc.scalar.copy(out=ag_st[:wt, w % 4, :], in_=pst[:wt, :D])
                if w % 4 == 3 or w == NW - 1:
                    flush_and_gather(0, w, wt, ag_st)
            fd_pass(0, fd_bufs[0], res_too=True)

            calls = L["calls"]
            nA = sum(1 for c in calls if c[1] == 0)

            for l in range(3):
                act_relu = l < 2
                res_src = res_dram if l == 0 else h_dram
                fd_cur = fd_bufs[l % 2]
                ag_st = None

                # ================= edge phase: pass A (seg 0), pass B (seg 1)
                cur_w = -1
                cur_sg = -1
                rst_ps = None
                for ci, (wi, sg, koff, gcs) in enumerate(calls):
                    if wi != cur_w or sg != cur_sg:
                        cur_w, cur_sg = wi, sg
                        rst_ps = ps.tile([P, 264], F32, tag="rstcat", space="PSUM")
                        fdw = sb2.tile([P, D], BF16, tag="fdw")
                        nc.sync.dma_start(out=fdw[:], in_=fd_cur[wi * P:wi * P + P, :])
                        if sg == 1:
                            # reload pass-A partial and seed the accumulator
                            prt = sb2.tile([P, 264], F32, tag="prt")
                            nc.sync.dma_start(out=prt[:],
                                              in_=part_dram[wi * P:(wi + 1) * P, :])
                            nc.tensor.matmul(rst_ps[:, :],
                                             lhsT=ident[:],
                                             rhs=prt[:],
                                             start=True, stop=False)
                            first_mm = False
                        else:
                            first_mm = True
                    # gather fs rows for up to 8 chunks per call
                    z8 = sb6.tile([P, 12, D], BF16, tag="z")
                    tab = tseg[l][sg]
                    nc.gpsimd.dma_gather(
                        z8[:, :gcs, :], tab[:, :], idx_t[:, koff * 8:koff * 8 + gcs * 8],
                        gcs * P, gcs * P, D, single_packet=False)
                    last_call = (ci + 1 == len(calls) or calls[ci + 1][0] != wi
                                 or calls[ci + 1][1] != sg)
                    for sub in range(0, gcs, 4):
                        gs = min(4, gcs - sub)
                        ko = koff + sub
                        z = z8[:, sub:sub + 4, :]
                        # one-hot (edges on partitions)
                        oh = sb.tile([P, 4, P], BF16, tag="oh")
                        nc.vector.tensor_tensor(
                            out=oh[:, :gs, :],
                            in0=dstw_t[:, ko:ko + gs].to_broadcast([P, gs, P]),
                            in1=iota_t[:].rearrange("p (g j) -> p g j", g=4)[:, :gs, :],
                            op=OP.is_equal)
                        # transposed one-hot (dst on partitions) via PE
                        ohT_ps = ps.tile([P, 4 * P], BF16, tag="ohT", space="PSUM")
                        for j in range(gs):
                            nc.tensor.transpose(out=ohT_ps[:, j * P:(j + 1) * P],
                                                in_=oh[:, j, :], identity=ident16[:])
                        ohT = sb.tile([P, 4 * P], BF16, tag="ohTs")
                        nc.scalar.copy(out=ohT[:, :gs * P], in_=ohT_ps[:, :gs * P])
                        # z_psum = OneHot_ve.T @ fdw + fs  (= fs[src]+fd[dst])
                        zps = ps.tile([P, 4, D], F32, tag="zps", space="PSUM")
                        for j in range(gs):
                            nc.tensor.matmul(zps[:, j, :], lhsT=ident16[:],
                                             rhs=z[:, j, :], start=True, stop=False)
                            nc.tensor.matmul(zps[:, j, :], lhsT=ohT[:, j * P:(j + 1) * P],
                                             rhs=fdw[:], start=False, stop=True)
                        # leaky-relu in one ACT pass: Prelu (parametric relu,
                        # same LUT set as Copy/Relu/Exp -> no table reloads)
                        lr = sb.tile([P, 4, D], BF16, tag="lr")
                        nc.scalar.activation(
                            lr[:, :gs, :].rearrange("p g d -> p (g d)"),
                            zps[:, :gs, :].rearrange("p g d -> p (g d)"),
                            AF.Prelu, alpha=SLOPE)
                        sm = sb.tile([P, 4, D], BF16, tag="sm")
                        nc.vector.tensor_tensor(
                            out=sm[:, :gs, :].rearrange("p g d -> p (g d)"),
                            in0=lr[:, :gs, :].rearrange("p g d -> p (g d)"),
                            in1=attn_t[l][:, :gs * D], op=OP.mult)
                        f1 = sb.tile([P, 4, 128], BF16, tag="f1")
                        nc.vector.tensor_tensor(
                            out=f1[:, :gs, :], in0=sm[:, :gs, 0:128],
                            in1=sm[:, :gs, 128:256], op=OP.add)
                        f2 = sb.tile([P, 4, 64], BF16, tag="f2")
                        nc.vector.tensor_tensor(
                            out=f2[:, :gs, :], in0=f1[:, :gs, 0:64],
                            in1=f1[:, :gs, 64:128], op=OP.add)
                        f3 = sb.tile([P, 4, 32], BF16, tag="f3")
                        nc.vector.tensor_tensor(
                            out=f3[:, :gs, :], in0=f2[:, :gs, 0:32],
                            in1=f2[:, :gs, 32:64], op=OP.add)
                        f4 = sb.tile([P, 4, 16], BF16, tag="f4")
                        nc.vector.tensor_tensor(
                            out=f4[:, :gs, :], in0=f3[:, :gs, 0:16],
                            in1=f3[:, :gs, 16:32], op=OP.add)
                        sc = sb.tile([P, 4, H], F32, tag="sc")
                        nc.vector.tensor_tensor(
                            out=sc[:, :gs, :], in0=f4[:, :gs, 0:8],
                            in1=f4[:, :gs, 8:16], op=OP.add)
                        wcat = sb.tile([P, 4, 264], BF16, tag="wcat")
                        nc.scalar.activation(wcat[:, :gs, D:D + H], sc[:, :gs, :], AF.Exp)
                        # W = ex * fs[src] (z8, bf16 SBUF; d-major keeps the
                        # last dim of every operand packed -> DVE fast mode)
                        nc.vector.tensor_tensor(
                            out=wcat[:, :gs, :D].rearrange("p g (d h) -> p g d h", h=H),
                            in0=z[:, :gs, :].rearrange("p g (d h) -> p g d h", h=H),
                            in1=wcat[:, :gs, D:D + H].unsqueeze(2).to_broadcast(
                                [P, gs, D // H, H]),
                            op=OP.mult)
                        # accumulate [rstU | denom]
                        for j in range(gs):
                            last = last_call and sub + gs >= gcs and j == gs - 1
                            nc.tensor.matmul(rst_ps[:, :], lhsT=oh[:, j, :],
                                             rhs=wcat[:, j, :], start=first_mm, stop=last)
                            first_mm = False
                    if not last_call:
                        continue
                    if sg == 0:
                        # pass A: spill partial [rstU|den] for this window
                        pt = sb2.tile([P, 264], F32, tag="pt")
                        nc.scalar.copy(out=pt[:], in_=rst_ps[:, :])
                        nc.scalar.dma_start(out=part_dram[wi * P:(wi + 1) * P, :],
                                            in_=pt[:])
                        continue
                    # ---- window epilogue (pass B)
                    wt = min(P, SH - wi * P)
                    den = sb2.tile([P, H], F32, tag="den")
                    nc.vector.tensor_scalar_max(den[:], rst_ps[:, D:D + H], 1e-30)
                    rec = sb2.tile([P, H], F32, tag="rec")
                    nc.vector.reciprocal(rec[:], den[:])
                    rn = sb2.tile([P, D], F32, tag="rn")
                    nc.vector.tensor_tensor(
                        out=rn[:].rearrange("p (d h) -> p d h", h=H),
                        in0=rst_ps[:, :D].rearrange("p (d h) -> p d h", h=H),
                        in1=rec[:].unsqueeze(1).to_broadcast([P, D // H, H]),
                        op=OP.mult)
                    rt = sb2.tile([P, D], F32, tag="rt")
                    nc.sync.dma_start(out=rt[:], in_=res_src[wi * P:wi * P + P, :])
                    nc.vector.tensor_tensor(out=rn[:], in0=rn[:], in1=rt[:], op=OP.add)
                    hsb = sb2.tile([P, D], F32, tag="hsb")
                    if act_relu:
                        nc.scalar.activation(hsb[:], rn[:], AF.Relu)
                    else:
                        nc.scalar.copy(out=hsb[:], in_=rn[:])
                    if l < 2:
                        nc.scalar.dma_start(out=h_dram[wi * P:wi * P + wt, :],
                                            in_=hsb[:wt, :])
                        for half in range(2):
                            tp = ps.tile([P, 4 * P], F32, tag="ohT", space="PSUM")
                            nc.tensor.transpose(out=tp[:, :P],
                                                in_=hsb[:, half * P:(half + 1) * P],
                                                identity=ident[:])
                            nc.scalar.copy(out=hT[:, half, wi * P:(wi + 1) * P],
                                           in_=tp[:, :P])
                        # interleaved next-layer fs GEMM for this window
                        if wi % 4 == 0:
                            ag_st = sb2.tile([P, 4, D], BF16, tag="agst")
                        pst = ps.tile([P, 264], F32, tag="ohT", space="PSUM")
                        gemm(lambda k: hT[:, k, wi * P:wi * P + wt], 2,
                             wsrc_t[l + 1], wt, pst)
                        nc.scalar.copy(out=ag_st[:wt, wi % 4, :], in_=pst[:wt, :D])
                        if wi % 4 == 3 or wi == NW - 1:
                            flush_and_gather(l + 1, wi, wt, ag_st)
                    else:
                        mean = sb2.tile([P, 32], F32, tag="mean")
                        nc.vector.tensor_reduce(
                            out=mean[:],
                            in_=hsb[:].rearrange("p (d h) -> p d h", h=H),
                            axis=AX.X, op=OP.add)
                        osb = sb2.tile([P, 32], F32, tag="osb")
                        nc.scalar.mul(osb[:], mean[:], 1.0 / H)
                        nc.scalar.dma_start(out=out_ext[wi * P:wi * P + wt, :],
                                            in_=osb[:wt, :])

                # fd GEMMs for the next layer (overlap the T2 collective)
                if l < 2:
                    fd_pass(l + 1, fd_bufs[(l + 1) % 2])
    nc.compile()
    return nc


# ---------------------------------------------------------------- host driver
def prep_inputs(features, src, dst, Wsrc1, Wdst1, attn1, Wres1,
                Wsrc2, Wdst2, attn2, Wsrc3, Wdst3, attn3):
    N, IN = features.shape
    L = build_layout(np.asarray(src), np.asarray(dst), N)
    featT = np.ascontiguousarray(np.asarray(features).T)
    SH = L["SH"]

    # d-major column order: new col j = (d, h) with j = d*8+h
    perm = np.array([(j % H) * 32 + j // H for j in range(256)])

    def attn_rep(a):
        flat = np.asarray(a).T.reshape(-1)  # [256] d-major
        return np.tile(np.tile(flat, 4)[None, :], (P, 1)).astype(np.float32)

    iota = np.tile(np.arange(P, dtype=np.float32)[None, :], (P, 4))
    ident = np.eye(P, dtype=np.float32)
    common = {
        "ident": ident, "iota4": iota,
        "Wsrc0": np.asarray(Wsrc1)[:, perm], "Wdst0": np.asarray(Wdst1)[:, perm],
        "Wres1": np.asarray(Wres1)[:, perm],
        "Wsrc1": np.asarray(Wsrc2)[perm][:, perm], "Wdst1": np.asarray(Wdst2)[perm][:, perm],
        "Wsrc2": np.asarray(Wsrc3)[perm][:, perm], "Wdst2": np.asarray(Wdst3)[perm][:, perm],
        "attn4_0": attn_rep(attn1), "attn4_1": attn_rep(attn2), "attn4_2": attn_rep(attn3),
    }
    in_maps = []
    for c in range(NCORE):
        m = dict(common)
        m["featT_loc"] = np.ascontiguousarray(featT[:, c * SH:(c + 1) * SH])
        m["idx_w"] = L["idx_w"][c]
        m["dstw"] = L["dstw_cols"][c]
        in_maps.append(m)
    return L, in_maps


_BUILD_CACHE = {}


def run(features, src, dst, Wsrc1, Wdst1, attn1, Wres1,
        Wsrc2, Wdst2, attn2, Wsrc3, Wdst3, attn3, trace=False):
    N, IN = features.shape
    L, in_maps = prep_inputs(features, src, dst, Wsrc1, Wdst1, attn1, Wres1,
                             Wsrc2, Wdst2, attn2, Wsrc3, Wdst3, attn3)
    key = (N, IN, L["NCH"])
    if key not in _BUILD_CACHE:
        _BUILD_CACHE[key] = build_kernel(N, IN, L)
    nc = _BUILD_CACHE[key]
    res = run_bass_kernel_spmd(nc, in_maps, list(range(NCORE)), trace=trace,
                               trace_cores=list(range(NCORE)) if trace else None)
    out = np.concatenate([res.results[c]["out"] for c in range(NCORE)], axis=0)
    return out, res


def kernel(features, src, dst,
           Wsrc1, Wdst1, attn1, b1, Wres1,
           Wsrc2, Wdst2, attn2, b2,
           Wsrc3, Wdst3, attn3, b3):
    """Full-input entry point. Biases are zeros in this model (asserted)."""
    for b in (b1, b2, b3):
        assert float(np.abs(np.asarray(b)).max()) == 0.0, "nonzero bias unsupported"
    out, _ = run(np.asarray(features, np.float32), np.asarray(src), np.asarray(dst),
                 Wsrc1, Wdst1, attn1, Wres1, Wsrc2, Wdst2, attn2,
                 Wsrc3, Wdst3, attn3)
    return out.astype(np.float32)


# revision 49
# speedup vs baseline: 1.6952x; 1.0196x over previous
"""GATv2 (3-layer, 8-head) on 8 Trainium2 NeuronCores.

Strategy (edge-parallel, dst-sharded):
- Core c owns destination nodes [c*N/8, (c+1)*N/8) and all edges into them.
- The per-layer fs table (all N nodes) is split into two SEGMENT tensors:
  T1 = rows [0, R1) of every core's shard, T2 = rows [R1, SH).  Each segment
  is AllGather'd separately (single collective per Shared tensor), and each
  has < 32768 rows so gather indices are plain int16.
- Host sorts each core's edges by (segment, dst-window), pads to 128-edge
  chunks with a chunk structure uniform across cores (SPMD: one program).
- Edge phase runs in two passes: pass A (seg-0 edges) accumulates partial
  [rstU|den] per window and spills to DRAM; pass B (seg-1) reloads the
  partial, finishes the window, and runs the epilogue.  Pass A only needs
  T1, so T2's AllGather hides under it.
- Next-layer fs GEMMs are interleaved into pass-B epilogues (flush groups of
  4 windows; T1/T2 collectives issued at the segment boundaries), and the
  next-layer fd GEMMs run after the edge phase, under the T2 collective.
- Edge math per 128-dst window: dma_gather fs[src] rows, expand fd[dst] via
  one-hot matmul, zps = fs[src]+fd[dst] in PSUM, lr = Prelu(zps) on ACT,
  score = reduce(lr*attn) on DVE, ex = exp(score), W = ex*fs[src] (bf16
  SBUF), rstU = OneHot @ [W | ex] on TensorE, rst = rstU/den + residual.
- Output: mean over heads, host concatenates the 8 dst shards.
"""
import sys
sys.path.insert(0, "/opt/trn_rl_repo")
import numpy as np
import concourse.bass as bass
import concourse.mybir as mybir
import concourse.tile as tile
from concourse import bacc
from concourse.bass_utils import run_bass_kernel_spmd

P = 128
NCORE = 8
SLOPE = 0.2
H = 8

F32 = mybir.dt.float32
R32 = mybir.dt.float32r
BF16 = mybir.dt.bfloat16
I16 = mybir.dt.int16
AX = mybir.AxisListType
OP = mybir.AluOpType
AF = mybir.ActivationFunctionType


def seg_split(SH, NW):
    """Segment boundary R1 (rows): window-aligned, both segments < 32768/8."""
    if NW >= 2:
        R1 = (NW // 2) * P
        R1 = max(R1, SH - 32768 // NCORE)
        R1 = min(R1, 32768 // NCORE)
    else:
        R1 = SH // 2
    return R1


# ---------------------------------------------------------------- host layout
def build_layout(src, dst, N):
    """Edge layout: per-core, segment-major then dst-window-sorted, 128-padded,
    chunk structure uniform across cores."""
    SH = N // NCORE
    NW = (SH + P - 1) // P
    SHP = NW * P
    R1 = seg_split(SHP if NW >= 2 else SH, NW)
    # per-core permutation: LPT-balance nodes into NW windows (cap P each)
    # by total in-degree, so per-(window) edge counts are near-uniform.
    import heapq
    pos_all = []
    for c in range(NCORE):
        m = (dst // SH) == c
        dl = dst[m] - c * SH
        deg = np.bincount(dl, minlength=SH)
        order = np.argsort(-deg, kind="stable")
        load = [(0, w, 0) for w in range(NW)]  # (edge load, window, used)
        heapq.heapify(load)
        pos = np.zeros(SH, np.int64)
        spill = []
        for v in order:
            while True:
                ld, w, used = heapq.heappop(load)
                if used < P:
                    break
                spill.append((ld, w, used))
            pos[v] = w * P + used
            heapq.heappush(load, (ld + int(deg[v]), w, used + 1))
            for it in spill:
                heapq.heappush(load, it)
            spill = []
        pos_all.append(pos)
    cores = []
    for c in range(NCORE):
        m = (dst // SH) == c
        s, d = src[m], dst[m]
        dl = pos_all[c][d - c * SH]          # permuted dst position
        w = dl // P
        # source-node table mapping: segment + row within segment table
        sc = s // SH
        srl_p = pos_all[0][0] * 0  # placeholder dtype
        srl = np.empty(len(s), np.int64)
        for cc in range(NCORE):
            mm = sc == cc
            srl[mm] = pos_all[cc][(s[mm] - cc * SH)]
        sg = (srl >= R1).astype(np.int64)
        trow = np.where(sg == 0, sc * R1 + srl, sc * (SHP - R1) + (srl - R1))
        order = np.lexsort((w, sg))
        s_t, dl, w, sg = trow[order], dl[order], w[order], sg[order]
        groups = {}
        for wi in range(NW):
            for g in range(2):
                gm = (w == wi) & (sg == g)
                groups[(wi, g)] = (s_t[gm], dl[gm])
        cores.append(groups)
    # uniform chunk counts per (window, segment)
    C = {}
    for wi in range(NW):
        for g in range(2):
            n = max(len(cores[c][(wi, g)][0]) for c in range(NCORE))
            C[(wi, g)] = (n + P - 1) // P
    NCH = sum(C.values())
    # flat edge arrays per core (memory order: window-major, segment inner)
    src_rel = np.zeros((NCORE, NCH * P), np.int16)
    dstw = np.full((NCORE, NCH * P), -1.0, np.float32)
    calls = []  # (window, segment, chunk_off, n_chunks)
    off = 0
    for wi in range(NW):
        for g in range(2):
            nch = C[(wi, g)]
            k = 0
            while k < nch:
                gs = min(12, nch - k)
                calls.append((wi, g, off + k, gs))
                k += gs
            for c in range(NCORE):
                s_t, dl = cores[c][(wi, g)]
                k0 = off * P
                src_rel[c, k0:k0 + len(s_t)] = s_t.astype(np.int16)
                dstw[c, k0:k0 + len(dl)] = (dl % P).astype(np.float32)
            off += nch
    assert off == NCH
    # segment-major call order: pass A (seg 0), then pass B (seg 1)
    calls = ([c for c in calls if c[1] == 0] + [c for c in calls if c[1] == 1])
    # wrapped int16 idx for dma_gather: per call, idx i -> [i%16, col+i//16]
    TOTC = NCH * P // 16
    idx_w = np.zeros((NCORE, P, TOTC), np.int16)
    for c in range(NCORE):
        w16 = src_rel[c].reshape(-1, 16).T  # [16, NCH*8]
        idx_w[c] = np.tile(w16, (8, 1))
    # chunk-column-major dstw: [P, NCH], edge k*P+p -> [p, k]
    dstw_cols = dstw.reshape(NCORE, NCH, P).transpose(0, 2, 1).copy()
    return dict(SH=SH, NW=NW, R1=R1, NCH=NCH, C=C, calls=calls,
                idx_w=idx_w, dstw_cols=dstw_cols, pos=pos_all)


# ---------------------------------------------------------------- bass kernel
def build_kernel(N, IN, L):
    """L = layout dict. IN = input feature dim (128)."""
    D = 256  # H*HID = H*OUT
    SH, NW, NCH, R1 = L["SH"], L["NW"], L["NCH"], L["R1"]
    SHP = NW * P                      # padded shard rows
    TOTC = NCH * P // 16
    S0, S1 = R1, SHP - R1             # segment sizes (per core)

    nc = bacc.Bacc("TRN2", target_bir_lowering=False, debug=False,
                   num_devices=NCORE)
    # ---- inputs (per-core where noted)
    featT_loc = nc.declare_dram_parameter("featT_loc", [IN, SHP], F32, isOutput=False)
    Ws = [nc.declare_dram_parameter(f"Wsrc{l}", [IN if l == 0 else D, D], F32, isOutput=False) for l in range(3)]
    Wd = [nc.declare_dram_parameter(f"Wdst{l}", [IN if l == 0 else D, D], F32, isOutput=False) for l in range(3)]
    Wres1 = nc.declare_dram_parameter("Wres1", [IN, D], F32, isOutput=False)
    attn4 = [nc.declare_dram_parameter(f"attn4_{l}", [P, 4 * D], F32, isOutput=False) for l in range(3)]
    iota4 = nc.declare_dram_parameter("iota4", [P, 4 * P], F32, isOutput=False)
    ident_in = nc.declare_dram_parameter("ident", [P, P], F32, isOutput=False)
    idx_in = nc.declare_dram_parameter("idx_w", [P, TOTC], I16, isOutput=False)   # per-core
    dstw_in = nc.declare_dram_parameter("dstw", [P, NCH], F32, isOutput=False)    # per-core
    out_ext = nc.declare_dram_parameter("out", [SHP, 32], F32, isOutput=True)

    with tile.TileContext(nc) as tc:
        with (
            tc.tile_pool(name="const", bufs=1) as cpool,
            tc.tile_pool(name="sbuf", bufs=4) as sb,
            tc.tile_pool(name="stage", bufs=2) as sbst,
            tc.tile_pool(name="sb6", bufs=5) as sb6,
            tc.tile_pool(name="sb2", bufs=3) as sb2,
            tc.tile_pool(name="psum", bufs=2, space="PSUM") as ps,
            tc.tile_pool(name="dram", bufs=1, space="DRAM") as dr,
        ):
            # ---- persistent constants
            ident = cpool.tile([P, P], F32)
            nc.sync.dma_start(out=ident[:], in_=ident_in[:, :])
            ident16 = cpool.tile([P, P], BF16, tag="ident16")
            nc.vector.tensor_copy(out=ident16[:], in_=ident[:])
            iota_t = cpool.tile([P, 4 * P], F32)
            nc.sync.dma_start(out=iota_t[:], in_=iota4[:, :])
            idx_t = cpool.tile([P, TOTC], I16)
            nc.sync.dma_start(out=idx_t[:], in_=idx_in[:, :])
            dstw_t = cpool.tile([P, NCH], F32)
            nc.sync.dma_start(out=dstw_t[:], in_=dstw_in[:, :])
            attn_t = []
            for l in range(3):
                a32 = cpool.tile([P, 4 * D], F32, tag=f"attn32_{l}")
                nc.sync.dma_start(out=a32[:], in_=attn4[l][:, :])
                a = cpool.tile([P, 4 * D], BF16, tag=f"attn{l}")
                nc.vector.tensor_copy(out=a[:], in_=a32[:])
                attn_t.append(a)
            hT = cpool.tile([P, 2, SHP], BF16, tag="hT")  # local shard, transposed

            # ---- DRAM internals
            # per-layer segment tables (AllGather destinations)
            tseg = [[dr.tile([NCORE * S0, D], BF16, tag=f"t1_{l}", name=f"t1_{l}",
                             addr_space="Shared"),
                     dr.tile([NCORE * S1, D], BF16, tag=f"t2_{l}", name=f"t2_{l}",
                             addr_space="Shared")] for l in range(3)]
            ag_inA = dr.tile([R1, D], BF16, tag="aginA")
            ag_inB = dr.tile([SHP - R1, D], BF16, tag="aginB")
            fd_bufs = [dr.tile([SHP, D], BF16, tag=f"fd{i}", name=f"fd{i}")
                       for i in range(2)]
            res_dram = dr.tile([SHP, D], F32, tag="res")
            h_dram = dr.tile([SHP, D], F32, tag="h")
            part_dram = dr.tile([SHP, 264], F32, tag="part")

            for _zi in range(5):
                zz = sb6.tile([P, 12, D], BF16, tag="z", name="zinit")
                nc.vector.memset(zz[:], 0.0)

            def gemm(lhsT_ap_fn, kchunks, rhs_t, mt, out_psum):
                """out_psum[mt, D] = sum_k lhsT_k.T @ rhs_k (bf16 fast path)"""
                for k in range(kchunks):
                    nc.tensor.matmul(out_psum[:mt, :D],
                                     lhsT=lhsT_ap_fn(k),
                                     rhs=rhs_t[:, k, :],
                                     start=(k == 0), stop=(k == kchunks - 1))

            def load_w(wparam, kchunks, tag):
                w32 = sbst.tile([P, kchunks, D], F32, tag="wload", name="w32")
                nc.sync.dma_start(
                    out=w32[:], in_=wparam.ap().rearrange("(c k) n -> k c n", k=P))
                wt = cpool.tile([P, kchunks, D], BF16, tag=tag, name=f"w_{tag}")
                nc.vector.tensor_copy(out=wt[:], in_=w32[:])
                return wt

            wsrc_t = [load_w(Ws[l], 1 if l == 0 else 2, f"wsrc{l}") for l in range(3)]
            wdst_t = [load_w(Wd[l], 1 if l == 0 else 2, f"wdst{l}") for l in range(3)]
            wres_t = load_w(Wres1, 1, "wres")

            def ag_store(r0, rows, src_ap):
                """Store staged GEMM rows [r0, r0+rows) into the right
                per-segment ag buffer (never crosses the R1 boundary)."""
                if r0 < R1:
                    dst_ap = ag_inA[r0:r0 + rows, :]
                else:
                    dst_ap = ag_inB[r0 - R1:r0 - R1 + rows, :]
                if rows % P == 0:
                    dst_ap = dst_ap.rearrange("(c p) d -> p c d", p=P)
                nc.scalar.dma_start(out=dst_ap, in_=src_ap)

            def flush_and_gather(l, wi, wt, ag_st):
                """Flush the 4-window staging group ending at window wi;
                issue segment collectives at the segment boundaries."""
                w0 = wi - wi % 4
                nwin = wi - w0 + 1
                r0 = w0 * P
                if NW < 2:  # tiny: single window straddles R1
                    ag_store(0, R1, ag_st[:R1, 0, :])
                    ag_store(R1, SHP - R1, ag_st[R1:SHP, 0, :])
                elif wt == P:
                    ag_store(r0, nwin * P, ag_st[:, :nwin, :])
                else:
                    if nwin > 1:
                        ag_store(r0, (nwin - 1) * P, ag_st[:, :nwin - 1, :])
                    ag_store(wi * P, wt, ag_st[:wt, nwin - 1, :])
                r1c = min((wi + 1) * P, SHP)
                if r1c == R1:  # segment-0 rows complete -> T1 collective
                    nc.gpsimd.collective_compute(
                        "AllGather", OP.bypass,
                        replica_groups=[list(range(NCORE))],
                        ins=[ag_inA.opt()], outs=[tseg[l][0].opt()])
                if r1c == SHP:  # all rows complete -> T2 collective
                    nc.gpsimd.collective_compute(
                        "AllGather", OP.bypass,
                        replica_groups=[list(range(NCORE))],
                        ins=[ag_inB.opt()], outs=[tseg[l][1].opt()])
                    if NW < 2:  # degenerate tiny case: T1 never hit above
                        nc.gpsimd.collective_compute(
                            "AllGather", OP.bypass,
                            replica_groups=[list(range(NCORE))],
                            ins=[ag_inA.opt()], outs=[tseg[l][0].opt()])

            GF = 8  # tiles per batched load/store group

            def fd_pass(l, dest, res_too=False):
                """fd (and layer-0 res) GEMMs from resident hT / featT_loc."""
                for g0 in range(0, NW, GF):
                    gn = min(GF, NW - g0)
                    if l == 0:
                        ft32 = sbst.tile([P, GF * P], F32, tag="ft")
                        cw = min(GF * P, SHP - g0 * P)
                        nc.sync.dma_start(out=ft32[:, :cw],
                                          in_=featT_loc[:, g0 * P:g0 * P + cw])
                        ft = sbst.tile([P, GF * P], BF16, tag="ftb")
                        nc.scalar.copy(out=ft[:, :cw], in_=ft32[:, :cw])
                    dests = [(wdst_t[l], dest, BF16)]
                    if res_too:
                        dests.append((wres_t, res_dram, F32))
                    for (rhs_t, dst_dram, dt_) in dests:
                        st = sbst.tile([P, GF, D], dt_,
                                       tag="fsst" if dt_ is BF16 else "fsst32")
                        for j in range(gn):
                            w = g0 + j
                            mt = min(P, SHP - w * P)
                            pst = ps.tile([P, 264], F32, tag="rstcat", space="PSUM")
                            if l == 0:
                                nc.tensor.matmul(
                                    pst[:mt, :D],
                                    lhsT=ft[:, j * P:j * P + mt],
                                    rhs=rhs_t[:, 0, :],
                                    start=True, stop=True)
                            else:
                                gemm(lambda k: hT[:, k, w * P:w * P + mt], 2,
                                     rhs_t, mt, pst)
                            nc.scalar.copy(out=st[:mt, j, :], in_=pst[:mt, :D])
                        rows = min(GF * P, SHP - g0 * P)
                        nfull = rows // P
                        r0 = g0 * P
                        if nfull:
                            nc.scalar.dma_start(
                                out=dst_dram[r0:r0 + nfull * P, :].rearrange(
                                    "(c p) d -> p c d", p=P),
                                in_=st[:, :nfull, :])
                        if rows % P:
                            nc.scalar.dma_start(
                                out=dst_dram[r0 + nfull * P:r0 + rows, :],
                                in_=st[:rows % P, nfull, :])

            # ================= layer 0 dense: sharded fs GEMM + collectives
            ag_st = None
            ft0 = None
            for w in range(NW):
                wt = min(P, SHP - w * P)
                if w % GF == 0:
                    ft32 = sbst.tile([P, GF * P], F32, tag="ft")
                    cw = min(GF * P, SHP - w * P)
                    nc.sync.dma_start(out=ft32[:, :cw],
                                      in_=featT_loc[:, w * P:w * P + cw])
                    ft0 = sbst.tile([P, GF * P], BF16, tag="ftb")
                    nc.scalar.copy(out=ft0[:, :cw], in_=ft32[:, :cw])
                if w % 4 == 0:
                    ag_st = sb2.tile([P, 4, D], BF16, tag="agst")
                pst = ps.tile([P, 264], F32, tag="rstcat", space="PSUM")
                nc.tensor.matmul(pst[:wt, :D],
                                 lhsT=ft0[:, (w % GF) * P:(w % GF) * P + wt],
                                 rhs=wsrc_t[0][:, 0, :],
                                 start=True, stop=True)
                nc.scalar.copy(out=ag_st[:wt, w % 4, :], in_=pst[:wt, :D])
                if w % 4 == 3 or w == NW - 1:
                    flush_and_gather(0, w, wt, ag_st)
            fd_pass(0, fd_bufs[0], res_too=True)

            calls = L["calls"]
            nA = sum(1 for c in calls if c[1] == 0)

            for l in range(3):
                act_relu = l < 2
                res_src = res_dram if l == 0 else h_dram
                fd_cur = fd_bufs[l % 2]
                ag_st = None

                # ================= edge phase: pass A (seg 0), pass B (seg 1)
                cur_w = -1
                cur_sg = -1
                rst_ps = None
                for ci, (wi, sg, koff, gcs) in enumerate(calls):
                    if wi != cur_w or sg != cur_sg:
                        cur_w, cur_sg = wi, sg
                        rst_ps = ps.tile([P, 264], F32, tag="rstcat", space="PSUM")
                        fdw = sb2.tile([P, D], BF16, tag="fdw")
                        nc.sync.dma_start(out=fdw[:], in_=fd_cur[wi * P:wi * P + P, :])
                        if sg == 1:
                            # reload pass-A partial and seed the accumulator
                            prt = sb2.tile([P, 264], F32, tag="prt")
                            nc.sync.dma_start(out=prt[:],
                                              in_=part_dram[wi * P:(wi + 1) * P, :])
                            nc.tensor.matmul(rst_ps[:, :],
                                             lhsT=ident[:],
                                             rhs=prt[:],
                                             start=True, stop=False)
                            first_mm = False
                        else:
                            first_mm = True
                    # gather fs rows for up to 8 chunks per call
                    z8 = sb6.tile([P, 12, D], BF16, tag="z")
                    tab = tseg[l][sg]
                    nc.gpsimd.dma_gather(
                        z8[:, :gcs, :], tab[:, :], idx_t[:, koff * 8:koff * 8 + gcs * 8],
                        gcs * P, gcs * P, D, single_packet=False)
                    last_call = (ci + 1 == len(calls) or calls[ci + 1][0] != wi
                                 or calls[ci + 1][1] != sg)
                    for sub in range(0, gcs, 4):
                        gs = min(4, gcs - sub)
                        ko = koff + sub
                        z = z8[:, sub:sub + 4, :]
                        # one-hot (edges on partitions)
                        oh = sb.tile([P, 4, P], BF16, tag="oh")
                        nc.vector.tensor_tensor(
                            out=oh[:, :gs, :],
                            in0=dstw_t[:, ko:ko + gs].to_broadcast([P, gs, P]),
                            in1=iota_t[:].rearrange("p (g j) -> p g j", g=4)[:, :gs, :],
                            op=OP.is_equal)
                        # transposed one-hot (dst on partitions) via PE
                        ohT_ps = ps.tile([P, 4 * P], BF16, tag="ohT", space="PSUM")
                        for j in range(gs):
                            nc.tensor.transpose(out=ohT_ps[:, j * P:(j + 1) * P],
                                                in_=oh[:, j, :], identity=ident16[:])
                        ohT = sb.tile([P, 4 * P], BF16, tag="ohTs")
                        nc.scalar.copy(out=ohT[:, :gs * P], in_=ohT_ps[:, :gs * P])
                        # z_psum = OneHot_ve.T @ fdw + fs  (= fs[src]+fd[dst])
                        zps = ps.tile([P, 4, D], F32, tag="zps", space="PSUM")
                        for j in range(gs):
                            nc.tensor.matmul(zps[:, j, :], lhsT=ident16[:],
                                             rhs=z[:, j, :], start=True, stop=False)
                            nc.tensor.matmul(zps[:, j, :], lhsT=ohT[:, j * P:(j + 1) * P],
                                             rhs=fdw[:], start=False, stop=True)
                        # leaky-relu in one ACT pass: Prelu (parametric relu,
                        # same LUT set as Copy/Relu/Exp -> no table reloads)
                        lr = sb.tile([P, 4, D], BF16, tag="lr")
                        nc.scalar.activation(
                            lr[:, :gs, :].rearrange("p g d -> p (g d)"),
                            zps[:, :gs, :].rearrange("p g d -> p (g d)"),
                            AF.Prelu, alpha=SLOPE)
                        sm = sb.tile([P, 4, D], BF16, tag="sm")
                        nc.vector.tensor_tensor(
                            out=sm[:, :gs, :].rearrange("p g d -> p (g d)"),
                            in0=lr[:, :gs, :].rearrange("p g d -> p (g d)"),
                            in1=attn_t[l][:, :gs * D], op=OP.mult)
                        f1 = sb.tile([P, 4, 128], BF16, tag="f1")
                        nc.vector.tensor_tensor(
                            out=f1[:, :gs, :], in0=sm[:, :gs, 0:128],
                            in1=sm[:, :gs, 128:256], op=OP.add)
                        f2 = sb.tile([P, 4, 64], BF16, tag="f2")
                        nc.vector.tensor_tensor(
                            out=f2[:, :gs, :], in0=f1[:, :gs, 0:64],
                            in1=f1[:, :gs, 64:128], op=OP.add)
                        f3 = sb.tile([P, 4, 32], BF16, tag="f3")
                        nc.vector.tensor_tensor(
                            out=f3[:, :gs, :], in0=f2[:, :gs, 0:32],
                            in1=f2[:, :gs, 32:64], op=OP.add)
                        f4 = sb.tile([P, 4, 16], BF16, tag="f4")
                        nc.vector.tensor_tensor(
                            out=f4[:, :gs, :], in0=f3[:, :gs, 0:16],
                            in1=f3[:, :gs, 16:32], op=OP.add)
                        sc = sb.tile([P, 4, H], F32, tag="sc")
                        nc.vector.tensor_tensor(
                            out=sc[:, :gs, :], in0=f4[:, :gs, 0:8],
                            in1=f4[:, :gs, 8:16], op=OP.add)
                        wcat = sb.tile([P, 4, 264], BF16, tag="wcat")
                        nc.scalar.activation(wcat[:, :gs, D:D + H], sc[:, :gs, :], AF.Exp)
                        # W = ex * fs[src] (z8, bf16 SBUF; d-major keeps the
                        # last dim of every operand packed -> DVE fast mode)
                        nc.vector.tensor_tensor(
                            out=wcat[:, :gs, :D].rearrange("p g (d h) -> p g d h", h=H),
                            in0=z[:, :gs, :].rearrange("p g (d h) -> p g d h", h=H),
                            in1=wcat[:, :gs, D:D + H].unsqueeze(2).to_broadcast(
                                [P, gs, D // H, H]),
                            op=OP.mult)
                        # accumulate [rstU | denom]
                        for j in range(gs):
                            last = last_call and sub + gs >= gcs and j == gs - 1
                            nc.tensor.matmul(rst_ps[:, :], lhsT=oh[:, j, :],
                                             rhs=wcat[:, j, :], start=first_mm, stop=last)
                            first_mm = False
                    if not last_call:
                        continue
                    if sg == 0:
                        # pass A: spill partial [rstU|den] for this window
                        pt = sb2.tile([P, 264], F32, tag="pt")
                        nc.scalar.copy(out=pt[:], in_=rst_ps[:, :])
                        nc.scalar.dma_start(out=part_dram[wi * P:(wi + 1) * P, :],
                                            in_=pt[:])
                        continue
                    # ---- window epilogue (pass B)
                    wt = min(P, SHP - wi * P)
                    den = sb2.tile([P, H], F32, tag="den")
                    nc.vector.tensor_scalar_max(den[:], rst_ps[:, D:D + H], 1e-30)
                    rec = sb2.tile([P, H], F32, tag="rec")
                    nc.vector.reciprocal(rec[:], den[:])
                    rn = sb2.tile([P, D], F32, tag="rn")
                    nc.vector.tensor_tensor(
                        out=rn[:].rearrange("p (d h) -> p d h", h=H),
                        in0=rst_ps[:, :D].rearrange("p (d h) -> p d h", h=H),
                        in1=rec[:].unsqueeze(1).to_broadcast([P, D // H, H]),
                        op=OP.mult)
                    rt = sb2.tile([P, D], F32, tag="rt")
                    nc.sync.dma_start(out=rt[:], in_=res_src[wi * P:wi * P + P, :])
                    nc.vector.tensor_tensor(out=rn[:], in0=rn[:], in1=rt[:], op=OP.add)
                    hsb = sb2.tile([P, D], F32, tag="hsb")
                    if act_relu:
                        nc.scalar.activation(hsb[:], rn[:], AF.Relu)
                    else:
                        nc.scalar.copy(out=hsb[:], in_=rn[:])
                    if l < 2:
                        nc.scalar.dma_start(out=h_dram[wi * P:wi * P + wt, :],
                                            in_=hsb[:wt, :])
                        for half in range(2):
                            tp = ps.tile([P, 4 * P], F32, tag="ohT", space="PSUM")
                            nc.tensor.transpose(out=tp[:, :P],
                                                in_=hsb[:, half * P:(half + 1) * P],
                                                identity=ident[:])
                            nc.scalar.copy(out=hT[:, half, wi * P:(wi + 1) * P],
                                           in_=tp[:, :P])
                        # interleaved next-layer fs GEMM for this window
                        if wi % 4 == 0:
                            ag_st = sb2.tile([P, 4, D], BF16, tag="agst")
                        pst = ps.tile([P, 264], F32, tag="ohT", space="PSUM")
                        gemm(lambda k: hT[:, k, wi * P:wi * P + wt], 2,
                             wsrc_t[l + 1], wt, pst)
                        nc.scalar.copy(out=ag_st[:wt, wi % 4, :], in_=pst[:wt, :D])
                        if wi % 4 == 3 or wi == NW - 1:
                            flush_and_gather(l + 1, wi, wt, ag_st)
                    else:
                        mean = sb2.tile([P, 32], F32, tag="mean")
                        nc.vector.tensor_reduce(
                            out=mean[:],
                            in_=hsb[:].rearrange("p (d h) -> p d h", h=H),
                            axis=AX.X, op=OP.add)
                        osb = sb2.tile([P, 32], F32, tag="osb")
                        nc.scalar.mul(osb[:], mean[:], 1.0 / H)
                        nc.scalar.dma_start(out=out_ext[wi * P:wi * P + wt, :],
                                            in_=osb[:wt, :])

                # fd GEMMs for the next layer (overlap the T2 collective)
                if l < 2:
                    fd_pass(l + 1, fd_bufs[(l + 1) % 2])
    nc.compile()
    return nc


# ---------------------------------------------------------------- host driver
def prep_inputs(features, src, dst, Wsrc1, Wdst1, attn1, Wres1,
                Wsrc2, Wdst2, attn2, Wsrc3, Wdst3, attn3):
    N, IN = features.shape
    L = build_layout(np.asarray(src), np.asarray(dst), N)
    featT = np.ascontiguousarray(np.asarray(features).T)
    SH = L["SH"]

    # d-major column order: new col j = (d, h) with j = d*8+h
    perm = np.array([(j % H) * 32 + j // H for j in range(256)])

    def attn_rep(a):
        flat = np.asarray(a).T.reshape(-1)  # [256] d-major
        return np.tile(np.tile(flat, 4)[None, :], (P, 1)).astype(np.float32)

    iota = np.tile(np.arange(P, dtype=np.float32)[None, :], (P, 4))
    ident = np.eye(P, dtype=np.float32)
    common = {
        "ident": ident, "iota4": iota,
        "Wsrc0": np.asarray(Wsrc1)[:, perm], "Wdst0": np.asarray(Wdst1)[:, perm],
        "Wres1": np.asarray(Wres1)[:, perm],
        "Wsrc1": np.asarray(Wsrc2)[perm][:, perm], "Wdst1": np.asarray(Wdst2)[perm][:, perm],
        "Wsrc2": np.asarray(Wsrc3)[perm][:, perm], "Wdst2": np.asarray(Wdst3)[perm][:, perm],
        "attn4_0": attn_rep(attn1), "attn4_1": attn_rep(attn2), "attn4_2": attn_rep(attn3),
    }
    SHP = L["NW"] * P
    in_maps = []
    for c in range(NCORE):
        m = dict(common)
        fl = np.zeros((featT.shape[0], SHP), np.float32)
        fl[:, L["pos"][c]] = featT[:, c * SH:(c + 1) * SH]
        m["featT_loc"] = fl
        m["idx_w"] = L["idx_w"][c]
        m["dstw"] = L["dstw_cols"][c]
        in_maps.append(m)
    return L, in_maps


_BUILD_CACHE = {}


def run(features, src, dst, Wsrc1, Wdst1, attn1, Wres1,
        Wsrc2, Wdst2, attn2, Wsrc3, Wdst3, attn3, trace=False):
    N, IN = features.shape
    L, in_maps = prep_inputs(features, src, dst, Wsrc1, Wdst1, attn1, Wres1,
                             Wsrc2, Wdst2, attn2, Wsrc3, Wdst3, attn3)
    key = (N, IN, L["NCH"])
    if key not in _BUILD_CACHE:
        _BUILD_CACHE[key] = build_kernel(N, IN, L)
    nc = _BUILD_CACHE[key]
    res = run_bass_kernel_spmd(nc, in_maps, list(range(NCORE)), trace=trace,
                               trace_cores=list(range(NCORE)) if trace else None)
    out = np.concatenate([res.results[c]["out"][L["pos"][c]]
                          for c in range(NCORE)], axis=0)
    return out, res


def kernel(features, src, dst,
           Wsrc1, Wdst1, attn1, b1, Wres1,
           Wsrc2, Wdst2, attn2, b2,
           Wsrc3, Wdst3, attn3, b3):
    """Full-input entry point. Biases are zeros in this model (asserted)."""
    for b in (b1, b2, b3):
        assert float(np.abs(np.asarray(b)).max()) == 0.0, "nonzero bias unsupported"
    out, _ = run(np.asarray(features, np.float32), np.asarray(src), np.asarray(dst),
                 Wsrc1, Wdst1, attn1, Wres1, Wsrc2, Wdst2, attn2,
                 Wsrc3, Wdst3, attn3)
    return out.astype(np.float32)
